# revision 1
# baseline (speedup 1.0000x reference)
"""CPCNet forward on 8 Trainium2 NeuronCores (Bass/Tile).

Data-parallel over batch: each of the 8 cores processes 16 of the 128
batch elements end-to-end (embed GEMM -> GRU over 16 context windows ->
bilinear scoring), parameters replicated. No collectives needed.

Per-core layout (all "transposed" space, embed dim on partitions):
  rows = flattened [C*T]-windows: Xc 256 (s*16+b), Xp 256 (s*16+b),
  Xb 2560 (nb*256 + s*16 + b).  ET[sbuf] = [100, 3072] embeddings^T.

Embed GEMM (the memory-bound bulk, ~103 MB/core, HW-measured ~370 us
wall for the whole net): X streams in natural layout [128 rows, k]
(fully contiguous DMA) and is cast f32->bf16 inside the SWDGE load DMAs;
PE transposes 128x128 bf16 blocks into PSUM (1 cyc/row vs 2-pass fp32);
DVE+ACT evacuate pairs of transposed chunks to SBUF; the PE accumulates
W_chunk.T @ X^T into E^T[100, 512] per 512-row block in bf16.

GRU + bilinear are fp32 and overlap the Xb embed stream (steps spread
between slabs; elementwise on the otherwise-idle GpSimd so the embed's
PSUM-evacuation copies never queue behind the GRU's serial chain).
Bilinear uses broadcast-multiply + ones-matmul column reduction to avoid
per-batch diagonal extraction; the final reduction runs as float32r.
"""

import numpy as np

import concourse.bacc as bacc
import concourse.mybir as mybir
import concourse.tile as tile
from concourse.bass_utils import run_bass_kernel_spmd

N_CORES = 8
BC = 16          # batch per core
NE = 16          # context windows (gru seq len)
NB = 10          # negative samples
CT = 8400        # flattened window (21*400)
E = 100          # embed dim == gru hidden
ROWS = BC * NE * (2 + NB)   # 3072 rows per core
NBLK = ROWS // 512          # 6 blocks of 512 rows
SLABS = [(8192, 208), (0, 2048), (2048, 2048), (4096, 2048), (6144, 2048)]
NCHUNK = 66                 # ceil(8400/128); last chunk is 80 wide

F32 = mybir.dt.float32
BF16 = mybir.dt.bfloat16

# The embed X pipeline runs in bf16: the f32->bf16 cast happens inside the
# SWDGE (gpsimd) load DMAs, so transposes and the embed matmul stream at
# 1 cyc/row on the PE (fp32 transposes measured 325 ns each = 515 us/core;
# bf16 ~3x cheaper).  HW-measured rel err of the bf16 embed ~2.4e-3.
# GRU + bilinear stay fp32.


def _block_src(Xc, Xp, Xb, blk, st, k0, kw):
    """DRAM source AP for 128-row subtile `st` of 512-row block `blk`,
    k-range [k0, k0+kw). Row order within subtile: (s, b), s-major."""
    if blk == 0:
        base = Xc if st < 2 else Xp
        sh = st % 2
        return base[:, sh * 8:(sh + 1) * 8, k0:k0 + kw].transpose([1, 0, 2])
    nb = 2 * (blk - 1) + st // 2
    sh = st % 2
    return Xb[:, sh * 8:(sh + 1) * 8, nb, k0:k0 + kw].transpose([1, 0, 2])


def _emit(nc, tc, ctx):
    Xc = nc.dram_tensor("Xc", [BC, NE, CT], F32, kind="ExternalInput").ap()
    Xp = nc.dram_tensor("Xp", [BC, NE, CT], F32, kind="ExternalInput").ap()
    Xb = nc.dram_tensor("Xb", [BC, NE, NB, CT], F32, kind="ExternalInput").ap()
    Wemb = nc.dram_tensor("Wemb", [128, NCHUNK * E], BF16,
                          kind="ExternalInput").ap()
    bemb = nc.dram_tensor("bemb", [E, 1], F32, kind="ExternalInput").ap()
    WihT = nc.dram_tensor("WihT", [E, 300], F32, kind="ExternalInput").ap()
    WhhT = nc.dram_tensor("WhhT", [E, 300], F32, kind="ExternalInput").ap()
    bias4 = nc.dram_tensor("bias4", [E, 4], F32, kind="ExternalInput").ap()
    Wbil = nc.dram_tensor("Wbil", [E, NE * E], F32, kind="ExternalInput").ap()
    ident = nc.dram_tensor("ident", [128, 128], BF16, kind="ExternalInput").ap()
    ones = nc.dram_tensor("ones", [E, 1], mybir.dt.float32r,
                          kind="ExternalInput").ap()
    out_d = nc.dram_tensor("out", [1, NE * BC * (NB + 1)], F32,
                           kind="ExternalOutput").ap()

    P = ctx.enter_context  # pools

    const = P(tc.tile_pool(name="const", bufs=1))
    xnat = P(tc.tile_pool(name="xnat", bufs=4))
    xtp = P(tc.tile_pool(name="xt", bufs=4))
    psT = P(tc.tile_pool(name="psT", bufs=3, space="PSUM"))
    psE = P(tc.tile_pool(name="psE", bufs=1, space="PSUM"))
    psS = P(tc.tile_pool(name="psS", bufs=1, space="PSUM"))
    small = P(tc.tile_pool(name="small", bufs=2))

    # ---- persistent SBUF ----
    # identity first: the very first transposes only need id_sb + one X slab
    id_sb = const.tile([128, 128], BF16)
    nc.sync.dma_start(id_sb[:], ident[:])
    # W_embed arrives pre-chunked [128, 66*100] and pre-cast to bf16 from
    # the host: one fully-contiguous 1.7 MB DMA, no on-chip cast, so the
    # first embed matmul is ready ~immediately.
    W_sb = const.tile([128, NCHUNK * E], BF16)
    nc.sync.dma_start(W_sb[:], Wemb[:])
    bemb_sb = const.tile([E, 1], F32)
    nc.scalar.dma_start(bemb_sb[:], bemb[:])
    WihT_sb = const.tile([E, 300], F32)
    nc.scalar.dma_start(WihT_sb[:], WihT[:])
    WhhT_sb = const.tile([E, 300], F32)
    nc.scalar.dma_start(WhhT_sb[:], WhhT[:])
    bias4_sb = const.tile([E, 4], F32)
    nc.scalar.dma_start(bias4_sb[:], bias4[:])
    Wbil_sb = const.tile([E, NE * E], F32)
    nc.scalar.dma_start(Wbil_sb[:], Wbil[:])
    ones_sb = const.tile([E, 1], mybir.dt.float32r)
    nc.scalar.dma_start(ones_sb[:], ones[:])

    ET = const.tile([E, ROWS], F32)                # all embeddings, transposed
    gi_sb = const.tile([E, NE * 3 * BC], F32)      # preacts, [s][r|z|n] blocks
    h = const.tile([E, BC], F32)                   # GRU hidden state (h^T)
    tmp_all = const.tile([E, NE * BC * (NB + 1)], mybir.dt.float32r)
    out_sb = const.tile([1, NE * BC * (NB + 1)], F32)

    gi_v = gi_sb.rearrange("e (s g b) -> e s g b", s=NE, g=3)

    def gru_init():
        # gi preacts for all 16 steps in 3 gate matmuls; biases folded
        # (r,z get b_ih+b_hh; n gets b_ih only).  Scattered into the
        # per-step-interleaved gi layout so each step reads one slice.
        nc.vector.memset(h[:], 0.0)
        for g in range(3):
            gp = psS.tile([E, NE * BC], F32, tag="sp0", name="gp")
            nc.tensor.matmul(gp[:, :], WihT_sb[:, g * E:(g + 1) * E],
                             ET[:, 0:NE * BC], start=True, stop=True)
            nc.scalar.add(gi_v[:, :, g, :],
                          gp.rearrange("e (s b) -> e s b", s=NE),
                          bias4_sb[:, g:g + 1])

    def gru_step(s):
        # DVE only evacuates gh (1 op); elementwise on the idle GpSimd,
        # sigmoid/tanh on ACT -- keeps the embed pair-copies from
        # head-of-line blocking behind the GRU's serial chain.
        c0 = s * 3 * BC
        gh = psS.tile([E, 3 * BC], F32, tag="sp1", name="gh")
        for g in range(3):
            nc.tensor.matmul(gh[:, g * BC:(g + 1) * BC],
                             WhhT_sb[:, g * E:(g + 1) * E], h[:],
                             start=True, stop=True)
        ghs = small.tile([E, 3 * BC], F32, tag="ghs", name="ghs")
        nc.vector.tensor_copy(ghs[:], gh[:])
        rzt = small.tile([E, 2 * BC], F32, tag="rzt", name="rzt")
        nc.gpsimd.tensor_add(rzt[:], ghs[:, 0:2 * BC], gi_sb[:, c0:c0 + 2 * BC])
        rz = small.tile([E, 2 * BC], F32, tag="rz", name="rz")
        nc.scalar.activation(rz[:], rzt[:],
                             mybir.ActivationFunctionType.Sigmoid)
        hn = small.tile([E, BC], F32, tag="hn", name="hn")
        nc.gpsimd.tensor_scalar_add(hn[:], ghs[:, 2 * BC:3 * BC],
                                    bias4_sb[:, 3:4])  # gh_n + b_hn
        t1 = small.tile([E, BC], F32, tag="t1", name="t1")
        nc.gpsimd.tensor_mul(t1[:], rz[:, 0:BC], hn[:])
        t2 = small.tile([E, BC], F32, tag="t2", name="t2")
        nc.gpsimd.tensor_add(t2[:], t1[:], gi_sb[:, c0 + 2 * BC:c0 + 3 * BC])
        n = small.tile([E, BC], F32, tag="n", name="n")
        nc.scalar.activation(n[:], t2[:], mybir.ActivationFunctionType.Tanh)
        d = small.tile([E, BC], F32, tag="d", name="d")
        nc.gpsimd.tensor_sub(d[:], h[:], n[:])
        zd = small.tile([E, BC], F32, tag="zd", name="zd")
        nc.gpsimd.tensor_mul(zd[:], rz[:, BC:2 * BC], d[:])
        nc.gpsimd.tensor_add(h[:], n[:], zd[:])    # h = n + z*(h-n)

    # ---- embed: 6 blocks of 512 rows; GRU interleaved after block 0 ----
    for blk in range(NBLK):
        et = psE.tile([E, 512], F32)
        nmm = 0
        for si, (k0, kw) in enumerate(SLABS):
            # one GRU step between slabs (blocks 2..5 handle steps 0..15;
            # block 1 runs gru_init emitted at the block-0 boundary)
            if 2 <= blk <= 5 and si < 4:
                gru_step(4 * (blk - 2) + si)
            xs = [xnat.tile([128, 2048], BF16, tag=f"xn{st}", name=f"xn{st}")
                  for st in range(4)]
            for st in range(4):
                # gpsimd SWDGE casts f32 -> bf16 in the DMA
                nc.gpsimd.dma_start(xs[st][:, 0:kw],
                                    _block_src(Xc, Xp, Xb, blk, st, k0, kw))
            nj = (kw + 127) // 128
            assert nj % 2 == 0
            jbase = k0 // 128
            for jp in range(nj // 2):
                pt = psT.tile([128, 1024], BF16)
                kjs = []
                for u in range(2):
                    j = jp * 2 + u
                    kj = min(128, kw - j * 128)
                    kjs.append(kj)
                    for st in range(4):
                        nc.tensor.transpose(
                            pt[0:kj, u * 512 + st * 128:u * 512 + (st + 1) * 128],
                            xs[st][:, j * 128:j * 128 + kj],
                            id_sb[:])
                xt = xtp.tile([128, 1024], BF16)
                if kjs[1] == 128:
                    nc.vector.tensor_copy(xt[:, 0:640], pt[:, 0:640])
                    nc.scalar.copy(xt[:, 640:1024], pt[:, 640:1024])
                else:  # last pair: u=1 chunk only has kjs[1] valid rows
                    nc.vector.tensor_copy(xt[:, 0:512], pt[:, 0:512])
                    nc.scalar.copy(xt[0:kjs[1], 512:1024], pt[0:kjs[1], 512:1024])
                for u in range(2):
                    jg = jbase + jp * 2 + u
                    nc.tensor.matmul(
                        et[:, :],
                        W_sb[0:kjs[u], jg * E:(jg + 1) * E],
                        xt[0:kjs[u], u * 512:u * 512 + 512],
                        start=(nmm == 0), stop=(nmm == NCHUNK - 1),
                        skip_group_check=True)
                    nmm += 1
        # bias + evacuate to ET
        nc.scalar.add(ET[:, blk * 512:(blk + 1) * 512], et[:, :],
                      bemb_sb[:, 0:1])
        # gi preacts as soon as block 0 (Ec) is done
        if blk == 0:
            gru_init()

    # ---- bilinear scores ----
    tmp_v = tmp_all.rearrange("e (s b p) -> e s b p", s=NE, b=BC)
    Eb_v = ET[:, 512:ROWS].rearrange("e (nb s b) -> e nb s b", nb=NB, s=NE)
    for s in range(NE):
        Ap = psS.tile([E, BC], F32, tag="bilA", name="Ap", bufs=2)
        nc.tensor.matmul(Ap[:, :], Wbil_sb[:, s * E:(s + 1) * E], h[:],
                         start=True, stop=True)  # A_s^T = W_bil[s].T @ h^T
        nc.vector.tensor_mul(tmp_v[:, s, :, 0],
                             ET[:, NE * BC + s * BC: NE * BC + (s + 1) * BC],
                             Ap[:])
        nc.vector.tensor_mul(
            tmp_v[:, s, :, 1:NB + 1].rearrange("e b p -> e p b"),
            Eb_v[:, :, s, :],
            Ap[:].unsqueeze(1).broadcast_to([E, NB, BC]))
    TOT = NE * BC * (NB + 1)
    for c0 in range(0, TOT, 512):
        w = min(512, TOT - c0)
        rp = psS.tile([1, 512], F32, tag="sp1")
        nc.tensor.matmul(rp[0:1, 0:w], ones_sb[:, 0:1], tmp_all[:, c0:c0 + w],
                         start=True, stop=True)
        nc.scalar.copy(out_sb[:, c0:c0 + w], rp[0:1, 0:w])
    nc.sync.dma_start(out_d[:], out_sb[:])


def build():
    import contextlib
    nc = bacc.Bacc("TRN2", target_bir_lowering=False, debug=False,
                   enable_asserts=False, num_devices=N_CORES)
    with tile.TileContext(nc) as tc:
        with contextlib.ExitStack() as ctx:
            _emit(nc, tc, ctx)
    nc.compile()
    return nc


_NC = None


def make_in_maps(Xc, Xp, Xb, W_embed, b_embed, W_ih, W_hh, b_ih, b_hh, W_bil):
    B = Xc.shape[0]
    Xc_r = np.ascontiguousarray(Xc, np.float32).reshape(B, NE, CT)
    Xp_r = np.ascontiguousarray(Xp, np.float32).reshape(B, NE, CT)
    Xb_r = np.ascontiguousarray(Xb, np.float32).reshape(B, NE, NB, CT)

    import ml_dtypes
    W_embed = np.ascontiguousarray(W_embed, np.float32)
    W_ch = np.zeros((128, NCHUNK * E), np.float32)
    for j in range(NCHUNK):
        kj = min(128, CT - j * 128)
        W_ch[:kj, j * E:(j + 1) * E] = W_embed[j * 128:j * 128 + kj]
    W_ch = W_ch.astype(ml_dtypes.bfloat16)
    bemb = np.ascontiguousarray(b_embed, np.float32).reshape(E, 1)
    WihT = np.ascontiguousarray(W_ih.T, np.float32)          # [100, 300]
    WhhT = np.ascontiguousarray(W_hh.T, np.float32)
    bias4 = np.stack([b_ih[0:E] + b_hh[0:E],
                      b_ih[E:2 * E] + b_hh[E:2 * E],
                      b_ih[2 * E:3 * E],
                      b_hh[2 * E:3 * E]], axis=1).astype(np.float32)
    Wbil_r = np.ascontiguousarray(
        np.transpose(W_bil, (1, 0, 2)).reshape(E, NE * E), np.float32)
    ident = np.eye(128).astype(ml_dtypes.bfloat16)
    ones = np.ones((E, 1), np.float32)

    shared = dict(Wemb=W_ch, bemb=bemb, WihT=WihT, WhhT=WhhT,
                  bias4=bias4, Wbil=Wbil_r, ident=ident, ones=ones)
    in_maps = []
    for c in range(N_CORES):
        sl = slice(c * BC, (c + 1) * BC)
        in_maps.append(dict(Xc=Xc_r[sl], Xp=Xp_r[sl], Xb=Xb_r[sl], **shared))
    return in_maps


def gather(results):
    outs = []
    for c in range(N_CORES):
        o = results[c]["out"].reshape(NE, BC, NB + 1)       # [s, b, p]
        outs.append(np.transpose(o, (1, 0, 2)))             # [b, s, p]
    return np.concatenate(outs, axis=0).astype(np.float32)  # [128, 16, 11]


def kernel(Xc, Xp, Xb, W_embed, b_embed, W_ih, W_hh, b_ih, b_hh, W_bil):
    global _NC
    if _NC is None:
        _NC = build()
    in_maps = make_in_maps(Xc, Xp, Xb, W_embed, b_embed, W_ih, W_hh,
                           b_ih, b_hh, W_bil)
    res = run_bass_kernel_spmd(_NC, in_maps, core_ids=list(range(N_CORES)))
    return gather(res.results)



# revision 2
# speedup vs baseline: 1.4137x; 1.4137x over previous
"""CPCNet forward on 8 Trainium2 NeuronCores (Bass/Tile).

Data-parallel over batch: each of the 8 cores processes 16 of the 128
batch elements end-to-end (embed GEMM -> GRU over 16 context windows ->
bilinear scoring), parameters replicated. No collectives needed.

The embed GEMM is memory-bound (~103 MB/core of f32 input). Two host-side
transforms cut the HW time:
  1. X is pre-cast to bf16 on the host: halves HBM traffic (~51.6 MB/core,
     DMA roofline ~144 us at 358 GB/s). The embed matmul was already bf16;
     numerics unchanged (rel err ~4e-3 vs fp64, tolerance 2e-2).
  2. X is pre-transposed on the host into k-major tiles [128 k-partitions,
     rows], so the kernel needs NO on-chip transposes: plain contiguous
     HWDGE DMAs feed the PE matmuls directly. (The old pipeline burned
     ~110 us of PE on 128x128 transposes plus DVE/ACT PSUM evacuations.)

Streams: Xcp (Xc+Xp, 512 rows, 8.6 MB) on the scalar HWDGE ring; Xb
(2560 rows, 43 MB) on the sync ring; params on the gpsimd SWDGE ring.
Per k-chunk j (66 of them), the PE accumulates W_j^T @ X_j^T into
6 persistent PSUM banks (1 Ecp + 5 Eb) over the whole stream. The GRU
(fp32) runs interleaved with the Xb stream as soon as Ecp lands;
bilinear scoring at the end reuses freed PSUM banks.

Row order inside E^T[100, 3072]: Xc 256 (s*16+b), Xp 256 (s*16+b),
Xb 2560 (nb*256 + s*16 + b) -- identical to the fp32 baseline, so the
GRU/bilinear code is carried over verbatim.
"""

import numpy as np

import concourse.bacc as bacc
import concourse.mybir as mybir
import concourse.tile as tile
from concourse.bass_utils import run_bass_kernel_spmd

N_CORES = 8
BC = 16          # batch per core
NE = 16          # context windows (gru seq len)
NB = 10          # negative samples
CT = 8400        # flattened window (21*400)
NK = 66          # k chunks of 128 (8400 zero-padded to 8448)
CTP = NK * 128
NT = NK // 2     # 33 dma tiles, 2 k-chunks each
E = 100          # embed dim == gru hidden
RCP = 2 * NE * BC          # 512 rows: Xc then Xp
RB = NB * NE * BC          # 2560 rows: nb*256 + s*16 + b
ROWS = RCP + RB            # 3072
NBB = RB // 512            # 5 Eb psum banks
TOT = NE * BC * (NB + 1)

F32 = mybir.dt.float32
BF16 = mybir.dt.bfloat16


def _emit(nc, tc, ctx):
    XcpT = nc.dram_tensor("XcpT", [NT, 128, 2 * RCP], BF16,
                          kind="ExternalInput").ap()
    XbT = nc.dram_tensor("XbT", [NT, 128, 2 * RB], BF16,
                         kind="ExternalInput").ap()
    Wemb = nc.dram_tensor("Wemb", [128, NK * E], BF16,
                          kind="ExternalInput").ap()
    bemb = nc.dram_tensor("bemb", [E, 1], F32, kind="ExternalInput").ap()
    WihT = nc.dram_tensor("WihT", [E, 300], F32, kind="ExternalInput").ap()
    WhhT = nc.dram_tensor("WhhT", [E, 300], F32, kind="ExternalInput").ap()
    bias4 = nc.dram_tensor("bias4", [E, 4], F32, kind="ExternalInput").ap()
    Wbil = nc.dram_tensor("Wbil", [E, NE * E], F32, kind="ExternalInput").ap()
    ones = nc.dram_tensor("ones", [E, 1], mybir.dt.float32r,
                          kind="ExternalInput").ap()
    out_d = nc.dram_tensor("out", [1, TOT], F32, kind="ExternalOutput").ap()

    P = ctx.enter_context  # pools
    const = P(tc.tile_pool(name="const", bufs=1))
    xcp = P(tc.tile_pool(name="xcp", bufs=3))
    xbp = P(tc.tile_pool(name="xbp", bufs=4))
    psE = P(tc.tile_pool(name="psE", bufs=2, space="PSUM"))  # ecp, Ap
    psB = P(tc.tile_pool(name="psB", bufs=1, space="PSUM"))  # 5 Eb banks
    psS = P(tc.tile_pool(name="psS", bufs=1, space="PSUM"))  # gp/gh/rp
    small = P(tc.tile_pool(name="small", bufs=2))

    # ---- persistent SBUF; params ride the gpsimd SWDGE ring so the two
    # HWDGE rings carry nothing but the X streams ----
    W_sb = const.tile([128, NK * E], BF16)
    nc.gpsimd.dma_start(W_sb[:], Wemb[:])
    bemb_sb = const.tile([E, 1], F32)
    nc.gpsimd.dma_start(bemb_sb[:], bemb[:])
    WihT_sb = const.tile([E, 300], F32)
    nc.gpsimd.dma_start(WihT_sb[:], WihT[:])
    WhhT_sb = const.tile([E, 300], F32)
    nc.gpsimd.dma_start(WhhT_sb[:], WhhT[:])
    bias4_sb = const.tile([E, 4], F32)
    nc.gpsimd.dma_start(bias4_sb[:], bias4[:])
    Wbil_sb = const.tile([E, NE * E], F32)
    nc.gpsimd.dma_start(Wbil_sb[:], Wbil[:])
    ones_sb = const.tile([E, 1], mybir.dt.float32r)
    nc.gpsimd.dma_start(ones_sb[:], ones[:])

    ET = const.tile([E, ROWS], F32)                # all embeddings, transposed
    gi_sb = const.tile([E, NE * 3 * BC], F32)      # preacts, [s][r|z|n] blocks
    h = const.tile([E, BC], F32)                   # GRU hidden state (h^T)
    tmp_all = const.tile([E, TOT], mybir.dt.float32r)
    out_sb = const.tile([1, TOT], F32)

    gi_v = gi_sb.rearrange("e (s g b) -> e s g b", s=NE, g=3)

    def gru_init():
        # gi preacts for all 16 steps in 3 gate matmuls; biases folded
        # (r,z get b_ih+b_hh; n gets b_ih only).
        nc.vector.memset(h[:], 0.0)
        for g in range(3):
            gp = psS.tile([E, NE * BC], F32, tag="sp", name="gp")
            nc.tensor.matmul(gp[:, :], WihT_sb[:, g * E:(g + 1) * E],
                             ET[:, 0:NE * BC], start=True, stop=True)
            nc.scalar.add(gi_v[:, :, g, :],
                          gp.rearrange("e (s b) -> e s b", s=NE),
                          bias4_sb[:, g:g + 1])

    def gru_step(s):
        # elementwise on GpSimd (otherwise idle), sigmoid/tanh on ACT,
        # DVE only evacuates gh from PSUM.
        c0 = s * 3 * BC
        gh = psS.tile([E, 3 * BC], F32, tag="sp", name="gh")
        for g in range(3):
            nc.tensor.matmul(gh[:, g * BC:(g + 1) * BC],
                             WhhT_sb[:, g * E:(g + 1) * E], h[:],
                             start=True, stop=True)
        ghs = small.tile([E, 3 * BC], F32, tag="ghs", name="ghs")
        nc.vector.tensor_copy(ghs[:], gh[:])
        rzt = small.tile([E, 2 * BC], F32, tag="rzt", name="rzt")
        nc.gpsimd.tensor_add(rzt[:], ghs[:, 0:2 * BC], gi_sb[:, c0:c0 + 2 * BC])
        rz = small.tile([E, 2 * BC], F32, tag="rz", name="rz")
        nc.scalar.activation(rz[:], rzt[:],
                             mybir.ActivationFunctionType.Sigmoid)
        hn = small.tile([E, BC], F32, tag="hn", name="hn")
        nc.gpsimd.tensor_scalar_add(hn[:], ghs[:, 2 * BC:3 * BC],
                                    bias4_sb[:, 3:4])  # gh_n + b_hn
        t1 = small.tile([E, BC], F32, tag="t1", name="t1")
        nc.gpsimd.tensor_mul(t1[:], rz[:, 0:BC], hn[:])
        t2 = small.tile([E, BC], F32, tag="t2", name="t2")
        nc.gpsimd.tensor_add(t2[:], t1[:], gi_sb[:, c0 + 2 * BC:c0 + 3 * BC])
        n = small.tile([E, BC], F32, tag="n", name="n")
        nc.scalar.activation(n[:], t2[:], mybir.ActivationFunctionType.Tanh)
        d = small.tile([E, BC], F32, tag="d", name="d")
        nc.gpsimd.tensor_sub(d[:], h[:], n[:])
        zd = small.tile([E, BC], F32, tag="zd", name="zd")
        nc.gpsimd.tensor_mul(zd[:], rz[:, BC:2 * BC], d[:])
        nc.gpsimd.tensor_add(h[:], n[:], zd[:])    # h = n + z*(h-n)

    # ---- Xcp stream (scalar HWDGE ring): 33 tiles of [128, 1024] ----
    ecp = psE.tile([E, RCP], F32, tag="pe", name="ecp")
    for t in range(NT):
        xt = xcp.tile([128, 2 * RCP], BF16, tag="xcp", name="xcp_t")
        nc.scalar.dma_start(xt[:], XcpT[t])
        for u in range(2):
            j = 2 * t + u
            nc.tensor.matmul(ecp[:, :], W_sb[:, j * E:(j + 1) * E],
                             xt[:, u * RCP:(u + 1) * RCP],
                             start=(j == 0), stop=(j == NK - 1),
                             skip_group_check=True)

    # ---- Xb stream (sync HWDGE ring): 33 tiles of [128, 5120];
    # GRU runs interleaved once Ecp has landed ----
    eb = [psB.tile([E, 512], F32, name=f"eb{i}") for i in range(NBB)]
    for t in range(NT):
        xt = xbp.tile([128, 2 * RB], BF16, tag="xb", name="xb_t")
        nc.sync.dma_start(xt[:], XbT[t])
        for u in range(2):
            j = 2 * t + u
            for blk in range(NBB):
                nc.tensor.matmul(eb[blk][:, :], W_sb[:, j * E:(j + 1) * E],
                                 xt[:, u * RB + blk * 512:
                                        u * RB + (blk + 1) * 512],
                                 start=(j == 0), stop=(j == NK - 1),
                                 skip_group_check=True)
        if t == 0:
            nc.scalar.add(ET[:, 0:RCP], ecp[:, :], bemb_sb[:, 0:1])
            gru_init()
        elif t <= NE:
            gru_step(t - 1)

    # ---- evacuate Eb banks ----
    for blk in range(NBB):
        nc.scalar.add(ET[:, RCP + blk * 512:RCP + (blk + 1) * 512],
                      eb[blk][:, :], bemb_sb[:, 0:1])

    # ---- bilinear scores ----
    tmp_v = tmp_all.rearrange("e (s b p) -> e s b p", s=NE, b=BC)
    Eb_v = ET[:, RCP:ROWS].rearrange("e (nb s b) -> e nb s b", nb=NB, s=NE)
    for s in range(NE):
        Ap = psE.tile([E, BC], F32, tag="pe", name="Ap")
        nc.tensor.matmul(Ap[:, :], Wbil_sb[:, s * E:(s + 1) * E], h[:],
                         start=True, stop=True)  # A_s^T = W_bil[s].T @ h^T
        nc.vector.tensor_mul(tmp_v[:, s, :, 0],
                             ET[:, NE * BC + s * BC: NE * BC + (s + 1) * BC],
                             Ap[:])
        nc.vector.tensor_mul(
            tmp_v[:, s, :, 1:NB + 1].rearrange("e b p -> e p b"),
            Eb_v[:, :, s, :],
            Ap[:].unsqueeze(1).broadcast_to([E, NB, BC]))
    for c0 in range(0, TOT, 512):
        w = min(512, TOT - c0)
        rp = psS.tile([1, 512], F32, tag="sp", name="rp")
        nc.tensor.matmul(rp[0:1, 0:w], ones_sb[:, 0:1], tmp_all[:, c0:c0 + w],
                         start=True, stop=True)
        nc.scalar.copy(out_sb[:, c0:c0 + w], rp[0:1, 0:w])
    nc.sync.dma_start(out_d[:], out_sb[:])


def build():
    import contextlib
    nc = bacc.Bacc("TRN2", target_bir_lowering=False, debug=False,
                   enable_asserts=False, num_devices=N_CORES)
    with tile.TileContext(nc) as tc:
        with contextlib.ExitStack() as ctx:
            _emit(nc, tc, ctx)
    nc.compile()
    return nc


_NC = None
_PREP = None


def _get_prep():
    """jax-cpu jitted reorder + bf16 cast + k-major transpose (XLA's blocked
    multithreaded transpose; a naive numpy transpose of 826 MB is too slow)."""
    global _PREP
    if _PREP is None:
        import jax
        import jax.numpy as jnp
        cpu = jax.devices("cpu")[0]

        def fmt(x, R):
            # [R, CT] -> [NT, 128, 2*R] bf16; tile t cols u*R+r hold
            # chunk j=2t+u, i.e. out[t, kk, u*R+r] = x[r, (2t+u)*128+kk]
            x = jnp.pad(x, ((0, 0), (0, CTP - CT))).astype(jnp.bfloat16)
            x = x.reshape(R, NT, 2, 128)
            return jnp.transpose(x, (1, 3, 2, 0)).reshape(NT, 128, 2 * R)

        def f(xc, xp, xb):
            # xc, xp: [BC, NE, CT]; xb: [BC, NE, NB, CT] (f32)
            xcp = jnp.concatenate([
                jnp.transpose(xc, (1, 0, 2)).reshape(NE * BC, CT),
                jnp.transpose(xp, (1, 0, 2)).reshape(NE * BC, CT)], axis=0)
            xbr = jnp.transpose(xb, (2, 1, 0, 3)).reshape(RB, CT)
            return fmt(xcp, RCP), fmt(xbr, RB)

        with jax.default_device(cpu):
            jf = jax.jit(f)

        def prep(xc, xp, xb):
            with jax.default_device(cpu):
                a, b = jf(xc, xp, xb)
                return np.asarray(a), np.asarray(b)
        _PREP = prep
    return _PREP


def make_in_maps(Xc, Xp, Xb, W_embed, b_embed, W_ih, W_hh, b_ih, b_hh, W_bil):
    B = Xc.shape[0]
    Xc_r = np.ascontiguousarray(Xc, np.float32).reshape(B, NE, CT)
    Xp_r = np.ascontiguousarray(Xp, np.float32).reshape(B, NE, CT)
    Xb_r = np.ascontiguousarray(Xb, np.float32).reshape(B, NE, NB, CT)

    import ml_dtypes
    W_embed = np.ascontiguousarray(W_embed, np.float32)
    W_ch = np.zeros((128, NK * E), np.float32)
    for j in range(NK):
        kj = min(128, CT - j * 128)
        W_ch[:kj, j * E:(j + 1) * E] = W_embed[j * 128:j * 128 + kj]
    W_ch = W_ch.astype(ml_dtypes.bfloat16)
    bemb = np.ascontiguousarray(b_embed, np.float32).reshape(E, 1)
    WihT = np.ascontiguousarray(W_ih.T, np.float32)          # [100, 300]
    WhhT = np.ascontiguousarray(W_hh.T, np.float32)
    bias4 = np.stack([b_ih[0:E] + b_hh[0:E],
                      b_ih[E:2 * E] + b_hh[E:2 * E],
                      b_ih[2 * E:3 * E],
                      b_hh[2 * E:3 * E]], axis=1).astype(np.float32)
    Wbil_r = np.ascontiguousarray(
        np.transpose(W_bil, (1, 0, 2)).reshape(E, NE * E), np.float32)
    ones = np.ones((E, 1), np.float32)

    shared = dict(Wemb=W_ch, bemb=bemb, WihT=WihT, WhhT=WhhT,
                  bias4=bias4, Wbil=Wbil_r, ones=ones)
    prep = _get_prep()
    in_maps = []
    for c in range(N_CORES):
        sl = slice(c * BC, (c + 1) * BC)
        xcpT, xbT = prep(Xc_r[sl], Xp_r[sl], Xb_r[sl])
        in_maps.append(dict(XcpT=xcpT, XbT=xbT, **shared))
    return in_maps


def gather(results):
    outs = []
    for c in range(N_CORES):
        o = results[c]["out"].reshape(NE, BC, NB + 1)       # [s, b, p]
        outs.append(np.transpose(o, (1, 0, 2)))             # [b, s, p]
    return np.concatenate(outs, axis=0).astype(np.float32)  # [128, 16, 11]


def kernel(Xc, Xp, Xb, W_embed, b_embed, W_ih, W_hh, b_ih, b_hh, W_bil):
    global _NC
    if _NC is None:
        _NC = build()
    in_maps = make_in_maps(Xc, Xp, Xb, W_embed, b_embed, W_ih, W_hh,
                           b_ih, b_hh, W_bil)
    res = run_bass_kernel_spmd(_NC, in_maps, core_ids=list(range(N_CORES)))
    return gather(res.results)


# revision 3
# speedup vs baseline: 1.6340x; 1.1558x over previous
"""CPCNet forward on 8 Trainium2 NeuronCores (Bass/Tile).

Data-parallel over batch: each of the 8 cores processes 16 of the 128
batch elements end-to-end (embed GEMM -> GRU over 16 context windows ->
bilinear scoring), parameters replicated. No collectives needed.

The embed GEMM is memory-bound (~103 MB/core of f32 input). Host-side
transforms cut the HW time:
  1. X is pre-cast to bf16 on the host: halves HBM traffic (~51.6 MB/core,
     DMA roofline ~125-144 us). The embed matmul was already bf16.
  2. X is pre-transposed on the host into k-major tiles [128 k-partitions,
     rows], so the kernel needs NO on-chip transposes: plain contiguous
     HWDGE DMAs feed the PE matmuls directly.
  3. W_embed chunks are zero-padded to M=128 output columns so the PE's
     automatic fast-weight-load kicks in (NumWeights==128) and LDWEIGHTS
     pipelines with the matmuls instead of serializing (~150ns/mm).

Single sync-ring DMA FIFO: Xcp (Xc+Xp, 512 rows, 4.3 MB) streams first at
full line rate, then Xb (2560 rows, 43 MB); params ride the gpsimd SWDGE
ring. Per k-chunk j (66), the PE accumulates W_j^T @ X_j^T into 6
persistent PSUM banks (1 Ecp + 5 Eb) over the whole stream. The GRU (fp32)
starts as soon as Ecp lands (~13 us) and interleaves one step per two Xb
tiles; its elementwise chain runs on the otherwise-idle Vector engine
(reading gh straight from PSUM) with sigmoid/tanh on ACT, so each step's
serial latency stays ~2-3 us and never blocks the PE queue.

Row order inside E^T[100, 3072]: Xc 256 (s*16+b), Xp 256 (s*16+b),
Xb 2560 (nb*256 + s*16 + b) -- identical to the fp32 baseline, so the
bilinear code is carried over verbatim.
"""

import numpy as np

import concourse.bacc as bacc
import concourse.mybir as mybir
import concourse.tile as tile
from concourse.bass_utils import run_bass_kernel_spmd

N_CORES = 8
BC = 16          # batch per core
NE = 16          # context windows (gru seq len)
NB = 10          # negative samples
CT = 8400        # flattened window (21*400)
NK = 66          # k chunks of 128 (8400 zero-padded to 8448)
CTP = NK * 128
NT = NK // 2     # 33 dma tiles, 2 k-chunks each
E = 100          # embed dim == gru hidden
RCP = 2 * NE * BC          # 512 rows: Xc then Xp
RB = NB * NE * BC          # 2560 rows: nb*256 + s*16 + b
ROWS = RCP + RB            # 3072
NBB = RB // 512            # 5 Eb psum banks
TOT = NE * BC * (NB + 1)

F32 = mybir.dt.float32
BF16 = mybir.dt.bfloat16


def _emit(nc, tc, ctx):
    XcpT = nc.dram_tensor("XcpT", [NT, 128, 2 * RCP], BF16,
                          kind="ExternalInput").ap()
    XbT = nc.dram_tensor("XbT", [NT, 128, 2 * RB], BF16,
                         kind="ExternalInput").ap()
    Wemb = nc.dram_tensor("Wemb", [128, NK * 128], BF16,
                          kind="ExternalInput").ap()
    bemb = nc.dram_tensor("bemb", [E, 1], F32, kind="ExternalInput").ap()
    WihT = nc.dram_tensor("WihT", [E, 300], F32, kind="ExternalInput").ap()
    WhhT = nc.dram_tensor("WhhT", [E, 300], F32, kind="ExternalInput").ap()
    bias4 = nc.dram_tensor("bias4", [E, 4], F32, kind="ExternalInput").ap()
    Wbil = nc.dram_tensor("Wbil", [E, NE * E], F32, kind="ExternalInput").ap()
    ones = nc.dram_tensor("ones", [E, 1], mybir.dt.float32r,
                          kind="ExternalInput").ap()
    out_d = nc.dram_tensor("out", [1, TOT], F32, kind="ExternalOutput").ap()

    P = ctx.enter_context  # pools
    const = P(tc.tile_pool(name="const", bufs=1))
    xcp = P(tc.tile_pool(name="xcp", bufs=4))
    xbp = P(tc.tile_pool(name="xbp", bufs=6))
    psE = P(tc.tile_pool(name="psE", bufs=2, space="PSUM"))  # ecp/gp/Ap
    psB = P(tc.tile_pool(name="psB", bufs=1, space="PSUM"))  # 5 Eb banks
    psS = P(tc.tile_pool(name="psS", bufs=1, space="PSUM"))  # gh/rp
    small = P(tc.tile_pool(name="small", bufs=2))

    # ---- persistent SBUF; params ride the gpsimd SWDGE ring so the sync
    # HWDGE ring carries nothing but the X stream ----
    W_sb = const.tile([128, NK * 128], BF16)
    nc.gpsimd.dma_start(W_sb[:], Wemb[:])
    bemb_sb = const.tile([E, 1], F32)
    nc.gpsimd.dma_start(bemb_sb[:], bemb[:])
    WihT_sb = const.tile([E, 300], F32)
    nc.gpsimd.dma_start(WihT_sb[:], WihT[:])
    WhhT_sb = const.tile([E, 300], F32)
    nc.gpsimd.dma_start(WhhT_sb[:], WhhT[:])
    bias4_sb = const.tile([E, 4], F32)
    nc.gpsimd.dma_start(bias4_sb[:], bias4[:])
    Wbil_sb = const.tile([E, NE * E], F32)
    nc.gpsimd.dma_start(Wbil_sb[:], Wbil[:])
    ones_sb = const.tile([E, 1], mybir.dt.float32r)
    nc.gpsimd.dma_start(ones_sb[:], ones[:])

    ET = const.tile([E, ROWS], F32)                # all embeddings, transposed
    gi_sb = const.tile([E, NE * 3 * BC], F32)      # preacts, [s][r|z|n] blocks
    h = const.tile([E, BC], F32)                   # GRU hidden state (h^T)
    tmp_all = const.tile([E, TOT], mybir.dt.float32r)
    out_sb = const.tile([1, TOT], F32)

    gi_v = gi_sb.rearrange("e (s g b) -> e s g b", s=NE, g=3)

    def gru_init():
        # gi preacts for all 16 steps in 3 gate matmuls; biases folded
        # (r,z get b_ih+b_hh; n gets b_ih only).
        nc.vector.memset(h[:], 0.0)
        for g in range(3):
            gp = psE.tile([E, NE * BC], F32, tag="pe", name="gp")
            nc.tensor.matmul(gp[:, :], WihT_sb[:, g * E:(g + 1) * E],
                             ET[:, 0:NE * BC], start=True, stop=True)
            nc.scalar.add(gi_v[:, :, g, :],
                          gp.rearrange("e (s b) -> e s b", s=NE),
                          bias4_sb[:, g:g + 1])

    def gru_step(s):
        # elementwise on the idle DVE, reading gh straight from PSUM;
        # sigmoid/tanh (and the b_hn broadcast add) on ACT.
        c0 = s * 3 * BC
        gh = psS.tile([E, 3 * BC], F32, tag="sp", name="gh")
        for g in range(3):
            nc.tensor.matmul(gh[:, g * BC:(g + 1) * BC],
                             WhhT_sb[:, g * E:(g + 1) * E], h[:],
                             start=True, stop=True)
        hn = small.tile([E, BC], F32, tag="hn", name="hn")
        nc.scalar.add(hn[:], gh[:, 2 * BC:3 * BC], bias4_sb[:, 3:4])
        rzt = small.tile([E, 2 * BC], F32, tag="rzt", name="rzt")
        nc.vector.tensor_add(rzt[:], gh[:, 0:2 * BC], gi_sb[:, c0:c0 + 2 * BC])
        rz = small.tile([E, 2 * BC], F32, tag="rz", name="rz")
        nc.scalar.activation(rz[:], rzt[:],
                             mybir.ActivationFunctionType.Sigmoid)
        t1 = small.tile([E, BC], F32, tag="t1", name="t1")
        nc.vector.tensor_mul(t1[:], rz[:, 0:BC], hn[:])
        t2 = small.tile([E, BC], F32, tag="t2", name="t2")
        nc.vector.tensor_add(t2[:], t1[:], gi_sb[:, c0 + 2 * BC:c0 + 3 * BC])
        n = small.tile([E, BC], F32, tag="n", name="n")
        nc.scalar.activation(n[:], t2[:], mybir.ActivationFunctionType.Tanh)
        d = small.tile([E, BC], F32, tag="d", name="d")
        nc.vector.tensor_sub(d[:], h[:], n[:])
        zd = small.tile([E, BC], F32, tag="zd", name="zd")
        nc.vector.tensor_mul(zd[:], rz[:, BC:2 * BC], d[:])
        nc.vector.tensor_add(h[:], n[:], zd[:])    # h = n + z*(h-n)

    # ---- Xcp stream: head of the sync FIFO, full line rate ----
    ecp = psE.tile([128, RCP], F32, tag="pe", name="ecp")
    for t in range(NT):
        xt = xcp.tile([128, 2 * RCP], BF16, tag="xcp", name="xcp_t")
        nc.sync.dma_start(xt[:], XcpT[t])
        for u in range(2):
            j = 2 * t + u
            nc.tensor.matmul(ecp[:, :], W_sb[:, j * 128:(j + 1) * 128],
                             xt[:, u * RCP:(u + 1) * RCP],
                             start=(j == 0), stop=(j == NK - 1),
                             skip_group_check=True)

    # ---- Xb stream (same FIFO, right behind); GRU interleaved:
    # one step per two tiles once Ecp has landed ----
    eb = [psB.tile([128, 512], F32, name=f"eb{i}") for i in range(NBB)]
    for t in range(NT):
        xt = xbp.tile([128, 2 * RB], BF16, tag="xb", name="xb_t")
        nc.sync.dma_start(xt[:], XbT[t])
        for u in range(2):
            j = 2 * t + u
            for blk in range(NBB):
                nc.tensor.matmul(eb[blk][:, :], W_sb[:, j * 128:(j + 1) * 128],
                                 xt[:, u * RB + blk * 512:
                                        u * RB + (blk + 1) * 512],
                                 start=(j == 0), stop=(j == NK - 1),
                                 skip_group_check=True)
        if t == 0:
            nc.scalar.add(ET[:, 0:RCP], ecp[0:E, :], bemb_sb[:, 0:1])
            gru_init()
        elif t % 2 == 0:
            gru_step(t // 2 - 1)

    # ---- evacuate Eb banks ----
    for blk in range(NBB):
        nc.scalar.add(ET[:, RCP + blk * 512:RCP + (blk + 1) * 512],
                      eb[blk][0:E, :], bemb_sb[:, 0:1])

    # ---- bilinear scores ----
    tmp_v = tmp_all.rearrange("e (s b p) -> e s b p", s=NE, b=BC)
    Eb_v = ET[:, RCP:ROWS].rearrange("e (nb s b) -> e nb s b", nb=NB, s=NE)
    for s in range(NE):
        Ap = psE.tile([E, BC], F32, tag="pe", name="Ap")
        nc.tensor.matmul(Ap[:, :], Wbil_sb[:, s * E:(s + 1) * E], h[:],
                         start=True, stop=True)  # A_s^T = W_bil[s].T @ h^T
        nc.vector.tensor_mul(tmp_v[:, s, :, 0],
                             ET[:, NE * BC + s * BC: NE * BC + (s + 1) * BC],
                             Ap[:])
        nc.vector.tensor_mul(
            tmp_v[:, s, :, 1:NB + 1].rearrange("e b p -> e p b"),
            Eb_v[:, :, s, :],
            Ap[:].unsqueeze(1).broadcast_to([E, NB, BC]))
    for c0 in range(0, TOT, 512):
        w = min(512, TOT - c0)
        rp = psS.tile([1, 512], F32, tag="sp", name="rp")
        nc.tensor.matmul(rp[0:1, 0:w], ones_sb[:, 0:1], tmp_all[:, c0:c0 + w],
                         start=True, stop=True)
        nc.scalar.copy(out_sb[:, c0:c0 + w], rp[0:1, 0:w])
    nc.sync.dma_start(out_d[:], out_sb[:])


def build():
    import contextlib
    nc = bacc.Bacc("TRN2", target_bir_lowering=False, debug=False,
                   enable_asserts=False, num_devices=N_CORES)
    with tile.TileContext(nc) as tc:
        with contextlib.ExitStack() as ctx:
            _emit(nc, tc, ctx)
    nc.compile()
    return nc


_NC = None
_PREP = None


def _get_prep():
    """jax-cpu jitted reorder + bf16 cast + k-major transpose (XLA's blocked
    multithreaded transpose; a naive numpy transpose of 826 MB is too slow)."""
    global _PREP
    if _PREP is None:
        import jax
        import jax.numpy as jnp
        cpu = jax.devices("cpu")[0]

        def fmt(x, R):
            # [R, CT] -> [NT, 128, 2*R] bf16; tile t cols u*R+r hold
            # chunk j=2t+u, i.e. out[t, kk, u*R+r] = x[r, (2t+u)*128+kk]
            x = jnp.pad(x, ((0, 0), (0, CTP - CT))).astype(jnp.bfloat16)
            x = x.reshape(R, NT, 2, 128)
            return jnp.transpose(x, (1, 3, 2, 0)).reshape(NT, 128, 2 * R)

        def f(xc, xp, xb):
            # xc, xp: [BC, NE, CT]; xb: [BC, NE, NB, CT] (f32)
            xcp = jnp.concatenate([
                jnp.transpose(xc, (1, 0, 2)).reshape(NE * BC, CT),
                jnp.transpose(xp, (1, 0, 2)).reshape(NE * BC, CT)], axis=0)
            xbr = jnp.transpose(xb, (2, 1, 0, 3)).reshape(RB, CT)
            return fmt(xcp, RCP), fmt(xbr, RB)

        with jax.default_device(cpu):
            jf = jax.jit(f)

        def prep(xc, xp, xb):
            with jax.default_device(cpu):
                a, b = jf(xc, xp, xb)
                return np.asarray(a), np.asarray(b)
        _PREP = prep
    return _PREP


def make_in_maps(Xc, Xp, Xb, W_embed, b_embed, W_ih, W_hh, b_ih, b_hh, W_bil):
    B = Xc.shape[0]
    Xc_r = np.ascontiguousarray(Xc, np.float32).reshape(B, NE, CT)
    Xp_r = np.ascontiguousarray(Xp, np.float32).reshape(B, NE, CT)
    Xb_r = np.ascontiguousarray(Xb, np.float32).reshape(B, NE, NB, CT)

    import ml_dtypes
    W_embed = np.ascontiguousarray(W_embed, np.float32)
    # chunk j at cols [j*128, j*128+E); M zero-padded to 128 so the PE's
    # fast-weight-load path (NumWeights==128) is enabled.
    W_ch = np.zeros((128, NK * 128), np.float32)
    for j in range(NK):
        kj = min(128, CT - j * 128)
        W_ch[:kj, j * 128:j * 128 + E] = W_embed[j * 128:j * 128 + kj]
    W_ch = W_ch.astype(ml_dtypes.bfloat16)
    bemb = np.ascontiguousarray(b_embed, np.float32).reshape(E, 1)
    WihT = np.ascontiguousarray(W_ih.T, np.float32)          # [100, 300]
    WhhT = np.ascontiguousarray(W_hh.T, np.float32)
    bias4 = np.stack([b_ih[0:E] + b_hh[0:E],
                      b_ih[E:2 * E] + b_hh[E:2 * E],
                      b_ih[2 * E:3 * E],
                      b_hh[2 * E:3 * E]], axis=1).astype(np.float32)
    Wbil_r = np.ascontiguousarray(
        np.transpose(W_bil, (1, 0, 2)).reshape(E, NE * E), np.float32)
    ones = np.ones((E, 1), np.float32)

    shared = dict(Wemb=W_ch, bemb=bemb, WihT=WihT, WhhT=WhhT,
                  bias4=bias4, Wbil=Wbil_r, ones=ones)
    prep = _get_prep()
    in_maps = []
    for c in range(N_CORES):
        sl = slice(c * BC, (c + 1) * BC)
        xcpT, xbT = prep(Xc_r[sl], Xp_r[sl], Xb_r[sl])
        in_maps.append(dict(XcpT=xcpT, XbT=xbT, **shared))
    return in_maps


def gather(results):
    outs = []
    for c in range(N_CORES):
        o = results[c]["out"].reshape(NE, BC, NB + 1)       # [s, b, p]
        outs.append(np.transpose(o, (1, 0, 2)))             # [b, s, p]
    return np.concatenate(outs, axis=0).astype(np.float32)  # [128, 16, 11]


def kernel(Xc, Xp, Xb, W_embed, b_embed, W_ih, W_hh, b_ih, b_hh, W_bil):
    global _NC
    if _NC is None:
        _NC = build()
    in_maps = make_in_maps(Xc, Xp, Xb, W_embed, b_embed, W_ih, W_hh,
                           b_ih, b_hh, W_bil)
    res = run_bass_kernel_spmd(_NC, in_maps, core_ids=list(range(N_CORES)))
    return gather(res.results)


# revision 4
# speedup vs baseline: 1.7991x; 1.1010x over previous
"""CPCNet forward on 8 Trainium2 NeuronCores (Bass/Tile).

Data-parallel over batch: each of the 8 cores processes 16 of the 128
batch elements end-to-end (embed GEMM -> GRU over 16 context windows ->
bilinear scoring), parameters replicated. No collectives needed.

The embed GEMM is memory-bound (~103 MB/core of f32 input). Host-side
transforms cut the HW time:
  1. X is pre-cast to bf16 on the host: halves HBM traffic (~51.6 MB/core,
     DMA roofline ~144 us at the 358 GB/s per-core HBM limit).
  2. X is pre-transposed on the host into k-major tiles [128 k-partitions,
     rows], so the kernel needs NO on-chip transposes: plain contiguous
     HWDGE DMAs feed the PE matmuls directly.
  3. W_embed chunks are zero-padded to M=128 so the PE's fast-weight-load
     path (NumWeights==128) pipelines LDWEIGHTS with the matmuls.

Single sync-ring DMA FIFO: Xcp (Xc+Xp, 4.3 MB, 33 tiles) streams first at
line rate, then Xb (43 MB) as 16 quad-chunk tiles of 2.5 MB + 1 pair tile
-- big tiles keep the 16 SDMA engines fed and give the PE a ~6 us budget
per tile. Per k-chunk j (66), the PE accumulates W_j^T @ X_j^T into 6
persistent PSUM banks (1 Ecp + 5 Eb). The GRU (fp32) starts once Ecp
lands (~18 us), one step per quad tile (two at tile 8) so h is final a
tile before the stream ends; its elementwise chain runs on the idle
Vector engine (reading gh straight from PSUM), sigmoid/tanh on ACT.
The bilinear A-matrix (16 tiny matmuls) and the positive-sample products
overlap the last tiles; the tail is just 5 PSUM evacuations, one batched
negative-sample multiply, and a pipelined ones-matmul reduction.

Row order inside E^T[100, 3072]: Xc 256 (s*16+b), Xp 256 (s*16+b),
Xb 2560 (nb*256 + s*16 + b). Output tmp/out layout is (s, p, b).
"""

import numpy as np

import concourse.bacc as bacc
import concourse.mybir as mybir
import concourse.tile as tile
from concourse.bass_utils import run_bass_kernel_spmd

N_CORES = 8
BC = 16          # batch per core
NE = 16          # context windows (gru seq len)
NB = 10          # negative samples
CT = 8400        # flattened window (21*400)
NK = 66          # k chunks of 128 (8400 zero-padded to 8448)
CTP = NK * 128
NT = NK // 2     # 33 Xcp dma tiles, 2 k-chunks each
NQ = 16          # Xb quad tiles (chunks 4t..4t+3); chunks 64,65 in a pair tile
E = 100          # embed dim == gru hidden
RCP = 2 * NE * BC          # 512 rows: Xc then Xp
RB = NB * NE * BC          # 2560 rows: nb*256 + s*16 + b
ROWS = RCP + RB            # 3072
NBB = RB // 512            # 5 Eb psum banks
TOT = NE * BC * (NB + 1)

F32 = mybir.dt.float32
BF16 = mybir.dt.bfloat16

# GRU steps run at Xb quad-tile boundaries; tile 8 carries two steps so
# all 16 finish by tile 15 and h is final before the pair tile streams.
_STEPS_AT = {t: [t - 1] for t in range(1, 8)}
_STEPS_AT[8] = [7, 8]
for _t in range(9, 16):
    _STEPS_AT[_t] = [_t]


def _emit(nc, tc, ctx):
    XcpT = nc.dram_tensor("XcpT", [NT, 128, 2 * RCP], BF16,
                          kind="ExternalInput").ap()
    XbT4 = nc.dram_tensor("XbT4", [NQ, 128, 4 * RB], BF16,
                          kind="ExternalInput").ap()
    XbT2 = nc.dram_tensor("XbT2", [128, 2 * RB], BF16,
                          kind="ExternalInput").ap()
    Wemb = nc.dram_tensor("Wemb", [128, NK * 128], BF16,
                          kind="ExternalInput").ap()
    bemb = nc.dram_tensor("bemb", [E, 1], F32, kind="ExternalInput").ap()
    WihT = nc.dram_tensor("WihT", [E, 300], F32, kind="ExternalInput").ap()
    WhhT = nc.dram_tensor("WhhT", [E, 300], F32, kind="ExternalInput").ap()
    bias4 = nc.dram_tensor("bias4", [E, 4], F32, kind="ExternalInput").ap()
    Wbil = nc.dram_tensor("Wbil", [E, NE * E], F32, kind="ExternalInput").ap()
    ones = nc.dram_tensor("ones", [E, 1], mybir.dt.float32r,
                          kind="ExternalInput").ap()
    out_d = nc.dram_tensor("out", [1, TOT], F32, kind="ExternalOutput").ap()

    P = ctx.enter_context  # pools
    const = P(tc.tile_pool(name="const", bufs=1))
    xcp = P(tc.tile_pool(name="xcp", bufs=6))
    xbp = P(tc.tile_pool(name="xbp", bufs=4))
    psE = P(tc.tile_pool(name="psE", bufs=2, space="PSUM"))  # ecp/gp/Aall/rp
    psB = P(tc.tile_pool(name="psB", bufs=1, space="PSUM"))  # 5 Eb banks
    psS = P(tc.tile_pool(name="psS", bufs=1, space="PSUM"))  # gh
    small = P(tc.tile_pool(name="small", bufs=2))

    # ---- persistent SBUF; params ride the gpsimd SWDGE ring so the sync
    # HWDGE ring carries nothing but the X stream ----
    W_sb = const.tile([128, NK * 128], BF16)
    nc.gpsimd.dma_start(W_sb[:], Wemb[:])
    bemb_sb = const.tile([E, 1], F32)
    nc.gpsimd.dma_start(bemb_sb[:], bemb[:])
    WihT_sb = const.tile([E, 300], F32)
    nc.gpsimd.dma_start(WihT_sb[:], WihT[:])
    WhhT_sb = const.tile([E, 300], F32)
    nc.gpsimd.dma_start(WhhT_sb[:], WhhT[:])
    bias4_sb = const.tile([E, 4], F32)
    nc.gpsimd.dma_start(bias4_sb[:], bias4[:])
    Wbil_sb = const.tile([E, NE * E], F32)
    nc.gpsimd.dma_start(Wbil_sb[:], Wbil[:])
    ones_sb = const.tile([E, 1], mybir.dt.float32r)
    nc.gpsimd.dma_start(ones_sb[:], ones[:])

    ET = const.tile([E, ROWS], F32)                # all embeddings, transposed
    gi_sb = const.tile([E, NE * 3 * BC], F32)      # preacts, [s][r|z|n] blocks
    h = const.tile([E, BC], F32)                   # GRU hidden state (h^T)
    A_sb = const.tile([E, NE * BC], F32)           # bilinear A_s^T, s-major
    tmp_all = const.tile([E, TOT], mybir.dt.float32r)   # (s, p, b) layout
    out_sb = const.tile([1, TOT], F32)

    gi_v = gi_sb.rearrange("e (s g b) -> e s g b", s=NE, g=3)

    def gru_init():
        # gi preacts for all 16 steps in 3 gate matmuls; biases folded
        # (r,z get b_ih+b_hh; n gets b_ih only).
        nc.vector.memset(h[:], 0.0)
        for g in range(3):
            gp = psE.tile([E, NE * BC], F32, tag="pe", name="gp")
            nc.tensor.matmul(gp[:, :], WihT_sb[:, g * E:(g + 1) * E],
                             ET[:, 0:NE * BC], start=True, stop=True)
            nc.scalar.add(gi_v[:, :, g, :],
                          gp.rearrange("e (s b) -> e s b", s=NE),
                          bias4_sb[:, g:g + 1])

    def gru_step(s):
        # elementwise on the idle DVE, reading gh straight from PSUM;
        # sigmoid/tanh (and the b_hn broadcast add) on ACT.
        c0 = s * 3 * BC
        gh = psS.tile([E, 3 * BC], F32, tag="sp", name="gh")
        for g in range(3):
            nc.tensor.matmul(gh[:, g * BC:(g + 1) * BC],
                             WhhT_sb[:, g * E:(g + 1) * E], h[:],
                             start=True, stop=True)
        hn = small.tile([E, BC], F32, tag="hn", name="hn")
        nc.scalar.add(hn[:], gh[:, 2 * BC:3 * BC], bias4_sb[:, 3:4])
        rzt = small.tile([E, 2 * BC], F32, tag="rzt", name="rzt")
        nc.vector.tensor_add(rzt[:], gh[:, 0:2 * BC], gi_sb[:, c0:c0 + 2 * BC])
        rz = small.tile([E, 2 * BC], F32, tag="rz", name="rz")
        nc.scalar.activation(rz[:], rzt[:],
                             mybir.ActivationFunctionType.Sigmoid)
        t1 = small.tile([E, BC], F32, tag="t1", name="t1")
        nc.vector.tensor_mul(t1[:], rz[:, 0:BC], hn[:])
        t2 = small.tile([E, BC], F32, tag="t2", name="t2")
        nc.vector.tensor_add(t2[:], t1[:], gi_sb[:, c0 + 2 * BC:c0 + 3 * BC])
        n = small.tile([E, BC], F32, tag="n", name="n")
        nc.scalar.activation(n[:], t2[:], mybir.ActivationFunctionType.Tanh)
        d = small.tile([E, BC], F32, tag="d", name="d")
        nc.vector.tensor_sub(d[:], h[:], n[:])
        zd = small.tile([E, BC], F32, tag="zd", name="zd")
        nc.vector.tensor_mul(zd[:], rz[:, BC:2 * BC], d[:])
        nc.vector.tensor_add(h[:], n[:], zd[:])    # h = n + z*(h-n)

    # ---- Xcp stream: head of the sync FIFO, full line rate ----
    ecp = psE.tile([128, RCP], F32, tag="pe", name="ecp")
    for t in range(NT):
        xt = xcp.tile([128, 2 * RCP], BF16, tag="xcp", name="xcp_t")
        nc.sync.dma_start(xt[:], XcpT[t])
        for u in range(2):
            j = 2 * t + u
            nc.tensor.matmul(ecp[:, :], W_sb[:, j * 128:(j + 1) * 128],
                             xt[:, u * RCP:(u + 1) * RCP],
                             start=(j == 0), stop=(j == NK - 1),
                             skip_group_check=True)

    # ---- Xb stream (same FIFO): 16 quad tiles + 1 pair tile ----
    eb = [psB.tile([128, 512], F32, name=f"eb{i}") for i in range(NBB)]

    def emb_mms(xt, j, un):
        for u in range(un):
            for blk in range(NBB):
                nc.tensor.matmul(eb[blk][:, :],
                                 W_sb[:, (j + u) * 128:(j + u + 1) * 128],
                                 xt[:, u * RB + blk * 512:
                                        u * RB + (blk + 1) * 512],
                                 start=(j + u == 0), stop=(j + u == NK - 1),
                                 skip_group_check=True)

    for t in range(NQ):
        xt = xbp.tile([128, 4 * RB], BF16, tag="xb", name="xb_t")
        nc.sync.dma_start(xt[:], XbT4[t])
        emb_mms(xt, 4 * t, 4)
        if t == 0:
            nc.scalar.add(ET[:, 0:RCP], ecp[0:E, :], bemb_sb[:, 0:1])
            gru_init()
        for s in _STEPS_AT.get(t, []):
            gru_step(s)

    # h is final: bilinear A (16 tiny matmuls) + positive products overlap
    # the pair tile's stream.
    A_ps = psE.tile([E, NE * BC], F32, tag="pe", name="A_ps")
    for s in range(NE):
        nc.tensor.matmul(A_ps[:, s * BC:(s + 1) * BC],
                         Wbil_sb[:, s * E:(s + 1) * E], h[:],
                         start=True, stop=True)  # A_s^T = W_bil[s].T @ h^T
    nc.scalar.copy(A_sb[:], A_ps[:])

    xt2 = xbp.tile([128, 2 * RB], BF16, tag="xb2", name="xb2_t")
    nc.sync.dma_start(xt2[:], XbT2[:])
    emb_mms(xt2, 64, 2)

    tmp_v = tmp_all.rearrange("e (s p b) -> e s p b", s=NE, p=NB + 1)
    A_v = A_sb.rearrange("e (s b) -> e s b", s=NE)
    nc.vector.tensor_mul(tmp_v[:, :, 0, :],
                         ET[:, NE * BC:2 * NE * BC]
                         .rearrange("e (s b) -> e s b", s=NE),
                         A_v)

    # ---- evacuate Eb banks (split ACT/DVE), batched negative products ----
    for blk in range(NBB):
        eng = nc.scalar if blk % 2 == 0 else nc.vector
        if eng is nc.scalar:
            nc.scalar.add(ET[:, RCP + blk * 512:RCP + (blk + 1) * 512],
                          eb[blk][0:E, :], bemb_sb[:, 0:1])
        else:
            nc.vector.tensor_scalar_add(
                ET[:, RCP + blk * 512:RCP + (blk + 1) * 512],
                eb[blk][0:E, :], bemb_sb[:, 0:1])
    Eb_v = ET[:, RCP:ROWS].rearrange("e (nb s b) -> e s nb b", nb=NB, s=NE)
    nc.vector.tensor_mul(tmp_v[:, :, 1:NB + 1, :], Eb_v,
                         A_v.unsqueeze(2).broadcast_to([E, NE, NB, BC]))

    # ---- ones-matmul reduction over E, pipelined via psE rotation ----
    for c0 in range(0, TOT, 512):
        w = min(512, TOT - c0)
        rp = psE.tile([1, 512], F32, tag="pe", name="rp")
        nc.tensor.matmul(rp[0:1, 0:w], ones_sb[:, 0:1], tmp_all[:, c0:c0 + w],
                         start=True, stop=True)
        nc.scalar.copy(out_sb[:, c0:c0 + w], rp[0:1, 0:w])
    nc.sync.dma_start(out_d[:], out_sb[:])


def build():
    import contextlib
    nc = bacc.Bacc("TRN2", target_bir_lowering=False, debug=False,
                   enable_asserts=False, num_devices=N_CORES)
    with tile.TileContext(nc) as tc:
        with contextlib.ExitStack() as ctx:
            _emit(nc, tc, ctx)
    nc.compile()
    return nc


_NC = None
_PREP = None


def _get_prep():
    """jax-cpu jitted reorder + bf16 cast + k-major transpose (XLA's blocked
    multithreaded transpose; a naive numpy transpose of 826 MB is too slow)."""
    global _PREP
    if _PREP is None:
        import jax
        import jax.numpy as jnp
        cpu = jax.devices("cpu")[0]

        def f(xc, xp, xb):
            # xc, xp: [BC, NE, CT]; xb: [BC, NE, NB, CT] (f32)
            xcp = jnp.concatenate([
                jnp.transpose(xc, (1, 0, 2)).reshape(NE * BC, CT),
                jnp.transpose(xp, (1, 0, 2)).reshape(NE * BC, CT)], axis=0)
            xcp = jnp.pad(xcp, ((0, 0), (0, CTP - CT))).astype(jnp.bfloat16)
            xcp = xcp.reshape(RCP, NT, 2, 128)
            xcpT = jnp.transpose(xcp, (1, 3, 2, 0)).reshape(NT, 128, 2 * RCP)

            xbr = jnp.transpose(xb, (2, 1, 0, 3)).reshape(RB, CT)
            xbr = jnp.pad(xbr, ((0, 0), (0, CTP - CT))).astype(jnp.bfloat16)
            x4 = xbr[:, :NQ * 4 * 128].reshape(RB, NQ, 4, 128)
            xbT4 = jnp.transpose(x4, (1, 3, 2, 0)).reshape(NQ, 128, 4 * RB)
            x2 = xbr[:, NQ * 4 * 128:].reshape(RB, 2, 128)
            xbT2 = jnp.transpose(x2, (2, 1, 0)).reshape(128, 2 * RB)
            return xcpT, xbT4, xbT2

        with jax.default_device(cpu):
            jf = jax.jit(f)

        def prep(xc, xp, xb):
            with jax.default_device(cpu):
                a, b4, b2 = jf(xc, xp, xb)
                return np.asarray(a), np.asarray(b4), np.asarray(b2)
        _PREP = prep
    return _PREP


def make_in_maps(Xc, Xp, Xb, W_embed, b_embed, W_ih, W_hh, b_ih, b_hh, W_bil):
    B = Xc.shape[0]
    Xc_r = np.ascontiguousarray(Xc, np.float32).reshape(B, NE, CT)
    Xp_r = np.ascontiguousarray(Xp, np.float32).reshape(B, NE, CT)
    Xb_r = np.ascontiguousarray(Xb, np.float32).reshape(B, NE, NB, CT)

    import ml_dtypes
    W_embed = np.ascontiguousarray(W_embed, np.float32)
    # chunk j at cols [j*128, j*128+E); M zero-padded to 128 so the PE's
    # fast-weight-load path (NumWeights==128) is enabled.
    W_ch = np.zeros((128, NK * 128), np.float32)
    for j in range(NK):
        kj = min(128, CT - j * 128)
        W_ch[:kj, j * 128:j * 128 + E] = W_embed[j * 128:j * 128 + kj]
    W_ch = W_ch.astype(ml_dtypes.bfloat16)
    bemb = np.ascontiguousarray(b_embed, np.float32).reshape(E, 1)
    WihT = np.ascontiguousarray(W_ih.T, np.float32)          # [100, 300]
    WhhT = np.ascontiguousarray(W_hh.T, np.float32)
    bias4 = np.stack([b_ih[0:E] + b_hh[0:E],
                      b_ih[E:2 * E] + b_hh[E:2 * E],
                      b_ih[2 * E:3 * E],
                      b_hh[2 * E:3 * E]], axis=1).astype(np.float32)
    Wbil_r = np.ascontiguousarray(
        np.transpose(W_bil, (1, 0, 2)).reshape(E, NE * E), np.float32)
    ones = np.ones((E, 1), np.float32)

    shared = dict(Wemb=W_ch, bemb=bemb, WihT=WihT, WhhT=WhhT,
                  bias4=bias4, Wbil=Wbil_r, ones=ones)
    prep = _get_prep()
    in_maps = []
    for c in range(N_CORES):
        sl = slice(c * BC, (c + 1) * BC)
        xcpT, xbT4, xbT2 = prep(Xc_r[sl], Xp_r[sl], Xb_r[sl])
        in_maps.append(dict(XcpT=xcpT, XbT4=xbT4, XbT2=xbT2, **shared))
    return in_maps


def gather(results):
    outs = []
    for c in range(N_CORES):
        o = results[c]["out"].reshape(NE, NB + 1, BC)       # [s, p, b]
        outs.append(np.transpose(o, (2, 0, 1)))             # [b, s, p]
    return np.concatenate(outs, axis=0).astype(np.float32)  # [128, 16, 11]


def kernel(Xc, Xp, Xb, W_embed, b_embed, W_ih, W_hh, b_ih, b_hh, W_bil):
    global _NC
    if _NC is None:
        _NC = build()
    in_maps = make_in_maps(Xc, Xp, Xb, W_embed, b_embed, W_ih, W_hh,
                           b_ih, b_hh, W_bil)
    res = run_bass_kernel_spmd(_NC, in_maps, core_ids=list(range(N_CORES)))
    return gather(res.results)


# revision 5
# speedup vs baseline: 1.8725x; 1.0408x over previous
"""CPCNet forward on 8 Trainium2 NeuronCores (Bass/Tile).

Data-parallel over batch: each of the 8 cores processes 16 of the 128
batch elements end-to-end (embed GEMM -> GRU over 16 context windows ->
bilinear scoring), parameters replicated. No collectives needed.

The embed GEMM is memory-bound (~103 MB/core of f32 input). Host-side
transforms cut the HW time:
  1. X is pre-cast to bf16 on the host: halves HBM traffic (~51.6 MB/core,
     DMA roofline ~144 us at the 358 GB/s per-core HBM limit).
  2. X is pre-transposed on the host into k-major tiles [128 k-partitions,
     rows]: plain contiguous HWDGE DMAs feed the PE matmuls directly.
  3. W_embed chunks are zero-padded to M=128 so the PE's fast-weight-load
     path (NumWeights==128) pipelines LDWEIGHTS with the matmuls.

Single sync-ring DMA FIFO: W (2.2 MB), then Xcp (Xc+Xp, 4.3 MB) as quad
tiles, then Xb (43 MB) as 16 quad-chunk tiles of 2.5 MB + 1 pair tile.
A burst of warmup matmuls on W data runs before the stream so the PE's
HAM throttle promotes to full clock before real matmuls arrive. Per
k-chunk j (66), the PE accumulates W_j^T @ X_j^T into 6 persistent PSUM
banks (1 Ecp + 5 Eb). The GRU (fp32) runs one step per quad tile (two at
tile 8) so h is final a tile early; its elementwise chain runs on the
idle Vector engine reading gh straight from PSUM, with a single 3-block
sigmoid on ACT (the 1-z factor comes free as sigmoid(-z_preact) via a
4th negated gate block prepared on the host). The bilinear A-matrix and
positive products overlap the last tiles; the tail is 5 bank
evacuations, two batched negative-product multiplies, and a pipelined
ones-matmul reduction.

Row order inside E^T[100, 3072]: Xc 256 (s*16+b), Xp 256 (s*16+b),
Xb 2560 (nb*256 + s*16 + b). Output tmp/out layout is (s, p, b).
"""

import numpy as np

import concourse.bacc as bacc
import concourse.mybir as mybir
import concourse.tile as tile
from concourse.bass_utils import run_bass_kernel_spmd

N_CORES = 8
BC = 16          # batch per core
NE = 16          # context windows (gru seq len)
NB = 10          # negative samples
CT = 8400        # flattened window (21*400)
NK = 66          # k chunks of 128 (8400 zero-padded to 8448)
CTP = NK * 128
NQ = 16          # quad tiles (chunks 4t..4t+3); chunks 64,65 in a pair tile
E = 100          # embed dim == gru hidden
RCP = 2 * NE * BC          # 512 rows: Xc then Xp
RB = NB * NE * BC          # 2560 rows: nb*256 + s*16 + b
ROWS = RCP + RB            # 3072
NBB = RB // 512            # 5 Eb psum banks
TOT = NE * BC * (NB + 1)

F32 = mybir.dt.float32
BF16 = mybir.dt.bfloat16

# GRU steps run at Xb quad-tile boundaries; tile 8 carries two steps so
# all 16 finish by tile 15 and h is final before the pair tile streams.
_STEPS_AT = {t: [t - 1] for t in range(1, 8)}
_STEPS_AT[8] = [7, 8]
for _t in range(9, 16):
    _STEPS_AT[_t] = [_t]


def _emit(nc, tc, ctx):
    XcpT4 = nc.dram_tensor("XcpT4", [NQ, 128, 4 * RCP], BF16,
                           kind="ExternalInput").ap()
    XcpT2 = nc.dram_tensor("XcpT2", [128, 2 * RCP], BF16,
                           kind="ExternalInput").ap()
    XbT4 = nc.dram_tensor("XbT4", [NQ, 128, 4 * RB], BF16,
                          kind="ExternalInput").ap()
    XbT2 = nc.dram_tensor("XbT2", [128, 2 * RB], BF16,
                          kind="ExternalInput").ap()
    Wemb = nc.dram_tensor("Wemb", [128, NK * 128], BF16,
                          kind="ExternalInput").ap()
    bemb = nc.dram_tensor("bemb", [E, 1], F32, kind="ExternalInput").ap()
    Wih4T = nc.dram_tensor("Wih4T", [E, 400], F32, kind="ExternalInput").ap()
    WhhT = nc.dram_tensor("WhhT", [E, 300], F32, kind="ExternalInput").ap()
    bias5 = nc.dram_tensor("bias5", [E, 5], F32, kind="ExternalInput").ap()
    Wbil = nc.dram_tensor("Wbil", [E, NE * E], F32, kind="ExternalInput").ap()
    ones = nc.dram_tensor("ones", [E, 1], mybir.dt.float32r,
                          kind="ExternalInput").ap()
    out_d = nc.dram_tensor("out", [1, TOT], F32, kind="ExternalOutput").ap()

    P = ctx.enter_context  # pools
    const = P(tc.tile_pool(name="const", bufs=1))
    xcp = P(tc.tile_pool(name="xcp", bufs=3))
    xbp = P(tc.tile_pool(name="xbp", bufs=4))
    psE = P(tc.tile_pool(name="psE", bufs=2, space="PSUM"))  # ecp/gp/Aall/rp
    psB = P(tc.tile_pool(name="psB", bufs=1, space="PSUM"))  # 5 Eb banks
    psS = P(tc.tile_pool(name="psS", bufs=1, space="PSUM"))  # gh
    small = P(tc.tile_pool(name="small", bufs=2))

    # W leads the sync FIFO (line rate, ~6us); other params ride gpsimd
    # SWDGE so the sync ring carries nothing but W + the X stream.
    W_sb = const.tile([128, NK * 128], BF16)
    nc.sync.dma_start(W_sb[:], Wemb[:])
    bemb_sb = const.tile([E, 1], F32)
    nc.gpsimd.dma_start(bemb_sb[:], bemb[:])
    Wih4T_sb = const.tile([E, 400], F32)
    nc.gpsimd.dma_start(Wih4T_sb[:], Wih4T[:])
    WhhT_sb = const.tile([E, 300], F32)
    nc.gpsimd.dma_start(WhhT_sb[:], WhhT[:])
    bias5_sb = const.tile([E, 5], F32)
    nc.gpsimd.dma_start(bias5_sb[:], bias5[:])
    Wbil_sb = const.tile([E, NE * E], F32)
    nc.gpsimd.dma_start(Wbil_sb[:], Wbil[:])
    ones_sb = const.tile([E, 1], mybir.dt.float32r)
    nc.gpsimd.dma_start(ones_sb[:], ones[:])

    ET = const.tile([E, ROWS], F32)                # all embeddings, transposed
    gi_sb = const.tile([E, NE * 4 * BC], F32)      # preacts, [s][r|z|n|mz]
    h = const.tile([E, BC], F32)                   # GRU hidden state (h^T)
    A_sb = const.tile([E, NE * BC], F32)           # bilinear A_s^T, s-major
    tmp_all = const.tile([E, TOT], mybir.dt.float32r)   # (s, p, b) layout
    out_sb = const.tile([1, TOT], F32)

    gi_v = gi_sb.rearrange("e (s g b) -> e s g b", s=NE, g=4)

    def gru_init():
        # gi preacts for all 16 steps in 4 gate matmuls (r, z, n, -z);
        # biases folded (r,z,mz get b_ih+b_hh; n gets b_ih only).
        nc.vector.memset(h[:], 0.0)
        for g in range(4):
            gp = psE.tile([E, NE * BC], F32, tag="pe", name="gp")
            nc.tensor.matmul(gp[:, :], Wih4T_sb[:, g * E:(g + 1) * E],
                             ET[:, 0:NE * BC], start=True, stop=True)
            nc.scalar.add(gi_v[:, :, g, :],
                          gp.rearrange("e (s b) -> e s b", s=NE),
                          bias5_sb[:, g:g + 1])

    def gru_step(s):
        # elementwise on the idle DVE reading gh straight from PSUM; one
        # 3-block sigmoid (r, z, 1-z) + one tanh on ACT.
        c0 = s * 4 * BC
        gh = psS.tile([E, 3 * BC], F32, tag="sp", name="gh")
        for g in range(3):
            nc.tensor.matmul(gh[:, g * BC:(g + 1) * BC],
                             WhhT_sb[:, g * E:(g + 1) * E], h[:],
                             start=True, stop=True)
        pre = small.tile([E, 4 * BC], F32, tag="pre", name="pre")
        nc.vector.tensor_add(pre[:, 0:2 * BC], gh[:, 0:2 * BC],
                             gi_sb[:, c0:c0 + 2 * BC])           # r,z preacts
        nc.vector.tensor_sub(pre[:, 2 * BC:3 * BC],
                             gi_sb[:, c0 + 3 * BC:c0 + 4 * BC],
                             gh[:, BC:2 * BC])                   # -(z preact)
        nc.vector.tensor_scalar_add(pre[:, 3 * BC:4 * BC],
                                    gh[:, 2 * BC:3 * BC],
                                    bias5_sb[:, 4:5])            # gh_n + b_hn
        sg = small.tile([E, 3 * BC], F32, tag="sg", name="sg")
        nc.scalar.activation(sg[:], pre[:, 0:3 * BC],
                             mybir.ActivationFunctionType.Sigmoid)
        t1 = small.tile([E, BC], F32, tag="t1", name="t1")
        nc.vector.tensor_mul(t1[:], sg[:, 0:BC], pre[:, 3 * BC:4 * BC])
        t2 = small.tile([E, BC], F32, tag="t2", name="t2")
        nc.vector.tensor_add(t2[:], t1[:],
                             gi_sb[:, c0 + 2 * BC:c0 + 3 * BC])
        zh = small.tile([E, BC], F32, tag="zh", name="zh")
        nc.vector.tensor_mul(zh[:], sg[:, BC:2 * BC], h[:])      # z*h_old
        n = small.tile([E, BC], F32, tag="n", name="n")
        nc.scalar.activation(n[:], t2[:], mybir.ActivationFunctionType.Tanh)
        on = small.tile([E, BC], F32, tag="on", name="on")
        nc.vector.tensor_mul(on[:], sg[:, 2 * BC:3 * BC], n[:])  # (1-z)*n
        nc.vector.tensor_add(h[:], on[:], zh[:])                 # new h

    # ---- PE warmup: ~4us of matmuls on W data so the HAM throttle
    # promotes to full clock before the real stream arrives ----
    eb = [psB.tile([128, 512], F32, name=f"eb{i}") for i in range(NBB)]
    for w in range(16):
        nc.tensor.matmul(eb[0][:, :], W_sb[:, (w % 4) * 128:(w % 4 + 1) * 128],
                         W_sb[:, 0:512], start=True, stop=True)

    # ---- Xcp stream (quad tiles, head of the FIFO after W) ----
    ecp = psE.tile([128, RCP], F32, tag="pe", name="ecp")

    def ecp_mms(xt, j, un):
        for u in range(un):
            nc.tensor.matmul(ecp[:, :], W_sb[:, (j + u) * 128:(j + u + 1) * 128],
                             xt[:, u * RCP:(u + 1) * RCP],
                             start=(j + u == 0), stop=(j + u == NK - 1),
                             skip_group_check=True)

    for t in range(NQ):
        xt = xcp.tile([128, 4 * RCP], BF16, tag="xcp", name="xcp_t")
        nc.sync.dma_start(xt[:], XcpT4[t])
        ecp_mms(xt, 4 * t, 4)
    xtc2 = xcp.tile([128, 2 * RCP], BF16, tag="xcp2", name="xcp2_t")
    nc.sync.dma_start(xtc2[:], XcpT2[:])
    ecp_mms(xtc2, 64, 2)

    # ---- Xb stream (same FIFO): 16 quad tiles + 1 pair tile ----
    def emb_mms(xt, j, un):
        for u in range(un):
            for blk in range(NBB):
                nc.tensor.matmul(eb[blk][:, :],
                                 W_sb[:, (j + u) * 128:(j + u + 1) * 128],
                                 xt[:, u * RB + blk * 512:
                                        u * RB + (blk + 1) * 512],
                                 start=(j + u == 0), stop=(j + u == NK - 1),
                                 skip_group_check=True)

    for t in range(NQ):
        xt = xbp.tile([128, 4 * RB], BF16, tag="xb", name="xb_t")
        nc.sync.dma_start(xt[:], XbT4[t])
        emb_mms(xt, 4 * t, 4)
        if t == 0:
            nc.scalar.add(ET[:, 0:RCP], ecp[0:E, :], bemb_sb[:, 0:1])
            gru_init()
        for s in _STEPS_AT.get(t, []):
            gru_step(s)

    # h is final: bilinear A (16 tiny matmuls) + positive products overlap
    # the pair tile's stream.
    A_ps = psE.tile([E, NE * BC], F32, tag="pe", name="A_ps")
    for s in range(NE):
        nc.tensor.matmul(A_ps[:, s * BC:(s + 1) * BC],
                         Wbil_sb[:, s * E:(s + 1) * E], h[:],
                         start=True, stop=True)  # A_s^T = W_bil[s].T @ h^T
    nc.scalar.copy(A_sb[:], A_ps[:])

    xt2 = xbp.tile([128, 2 * RB], BF16, tag="xb2", name="xb2_t")
    nc.sync.dma_start(xt2[:], XbT2[:])
    emb_mms(xt2, 64, 2)

    tmp_v = tmp_all.rearrange("e (s p b) -> e s p b", s=NE, p=NB + 1)
    A_v = A_sb.rearrange("e (s b) -> e s b", s=NE)
    nc.vector.tensor_mul(tmp_v[:, :, 0, :],
                         ET[:, NE * BC:2 * NE * BC]
                         .rearrange("e (s b) -> e s b", s=NE),
                         A_v)

    # ---- evacuate Eb banks (ACT for 0-2, DVE for 3-4), then the
    # negative products in two batched multiplies (nb 0-3, nb 4-9) ----
    for blk in range(3):
        nc.scalar.add(ET[:, RCP + blk * 512:RCP + (blk + 1) * 512],
                      eb[blk][0:E, :], bemb_sb[:, 0:1])
    for blk in range(3, NBB):
        nc.vector.tensor_scalar_add(
            ET[:, RCP + blk * 512:RCP + (blk + 1) * 512],
            eb[blk][0:E, :], bemb_sb[:, 0:1])
    Eb_v = ET[:, RCP:ROWS].rearrange("e (nb s b) -> e s nb b", nb=NB, s=NE)
    nc.vector.tensor_mul(tmp_v[:, :, 1:5, :], Eb_v[:, :, 0:4, :],
                         A_v.unsqueeze(2).broadcast_to([E, NE, 4, BC]))
    nc.vector.tensor_mul(tmp_v[:, :, 5:NB + 1, :], Eb_v[:, :, 4:NB, :],
                         A_v.unsqueeze(2).broadcast_to([E, NE, 6, BC]))

    # ---- ones-matmul reduction over E, pipelined via psE rotation ----
    for c0 in range(0, TOT, 512):
        w = min(512, TOT - c0)
        rp = psE.tile([1, 512], F32, tag="pe", name="rp")
        nc.tensor.matmul(rp[0:1, 0:w], ones_sb[:, 0:1], tmp_all[:, c0:c0 + w],
                         start=True, stop=True)
        nc.scalar.copy(out_sb[:, c0:c0 + w], rp[0:1, 0:w])
    nc.sync.dma_start(out_d[:], out_sb[:])


def build():
    import contextlib
    nc = bacc.Bacc("TRN2", target_bir_lowering=False, debug=False,
                   enable_asserts=False, num_devices=N_CORES)
    with tile.TileContext(nc) as tc:
        with contextlib.ExitStack() as ctx:
            _emit(nc, tc, ctx)
    nc.compile()
    return nc


_NC = None
_PREP = None


def _get_prep():
    """jax-cpu jitted reorder + bf16 cast + k-major transpose (XLA's blocked
    multithreaded transpose; a naive numpy transpose of 826 MB is too slow)."""
    global _PREP
    if _PREP is None:
        import jax
        import jax.numpy as jnp
        cpu = jax.devices("cpu")[0]

        def fmt(x, R):
            x = jnp.pad(x, ((0, 0), (0, CTP - CT))).astype(jnp.bfloat16)
            x4 = x[:, :NQ * 4 * 128].reshape(R, NQ, 4, 128)
            xT4 = jnp.transpose(x4, (1, 3, 2, 0)).reshape(NQ, 128, 4 * R)
            x2 = x[:, NQ * 4 * 128:].reshape(R, 2, 128)
            xT2 = jnp.transpose(x2, (2, 1, 0)).reshape(128, 2 * R)
            return xT4, xT2

        def f(xc, xp, xb):
            # xc, xp: [BC, NE, CT]; xb: [BC, NE, NB, CT] (f32)
            xcp = jnp.concatenate([
                jnp.transpose(xc, (1, 0, 2)).reshape(NE * BC, CT),
                jnp.transpose(xp, (1, 0, 2)).reshape(NE * BC, CT)], axis=0)
            xbr = jnp.transpose(xb, (2, 1, 0, 3)).reshape(RB, CT)
            return fmt(xcp, RCP) + fmt(xbr, RB)

        with jax.default_device(cpu):
            jf = jax.jit(f)

        def prep(xc, xp, xb):
            with jax.default_device(cpu):
                return tuple(np.asarray(v) for v in jf(xc, xp, xb))
        _PREP = prep
    return _PREP


def make_in_maps(Xc, Xp, Xb, W_embed, b_embed, W_ih, W_hh, b_ih, b_hh, W_bil):
    B = Xc.shape[0]
    Xc_r = np.ascontiguousarray(Xc, np.float32).reshape(B, NE, CT)
    Xp_r = np.ascontiguousarray(Xp, np.float32).reshape(B, NE, CT)
    Xb_r = np.ascontiguousarray(Xb, np.float32).reshape(B, NE, NB, CT)

    import ml_dtypes
    W_embed = np.ascontiguousarray(W_embed, np.float32)
    # chunk j at cols [j*128, j*128+E); M zero-padded to 128 so the PE's
    # fast-weight-load path (NumWeights==128) is enabled.
    W_ch = np.zeros((128, NK * 128), np.float32)
    for j in range(NK):
        kj = min(128, CT - j * 128)
        W_ch[:kj, j * 128:j * 128 + E] = W_embed[j * 128:j * 128 + kj]
    W_ch = W_ch.astype(ml_dtypes.bfloat16)
    bemb = np.ascontiguousarray(b_embed, np.float32).reshape(E, 1)
    WihT = np.ascontiguousarray(W_ih.T, np.float32)          # [100, 300]
    Wih4T = np.concatenate([WihT, -WihT[:, E:2 * E]], axis=1)  # [100, 400]
    WhhT = np.ascontiguousarray(W_hh.T, np.float32)
    bz = b_ih[E:2 * E] + b_hh[E:2 * E]
    bias5 = np.stack([b_ih[0:E] + b_hh[0:E], bz, b_ih[2 * E:3 * E],
                      -bz, b_hh[2 * E:3 * E]], axis=1).astype(np.float32)
    Wbil_r = np.ascontiguousarray(
        np.transpose(W_bil, (1, 0, 2)).reshape(E, NE * E), np.float32)
    ones = np.ones((E, 1), np.float32)

    shared = dict(Wemb=W_ch, bemb=bemb, Wih4T=Wih4T, WhhT=WhhT,
                  bias5=bias5, Wbil=Wbil_r, ones=ones)
    prep = _get_prep()
    in_maps = []
    for c in range(N_CORES):
        sl = slice(c * BC, (c + 1) * BC)
        xcpT4, xcpT2, xbT4, xbT2 = prep(Xc_r[sl], Xp_r[sl], Xb_r[sl])
        in_maps.append(dict(XcpT4=xcpT4, XcpT2=xcpT2, XbT4=xbT4, XbT2=xbT2,
                            **shared))
    return in_maps


def gather(results):
    outs = []
    for c in range(N_CORES):
        o = results[c]["out"].reshape(NE, NB + 1, BC)       # [s, p, b]
        outs.append(np.transpose(o, (2, 0, 1)))             # [b, s, p]
    return np.concatenate(outs, axis=0).astype(np.float32)  # [128, 16, 11]


def kernel(Xc, Xp, Xb, W_embed, b_embed, W_ih, W_hh, b_ih, b_hh, W_bil):
    global _NC
    if _NC is None:
        _NC = build()
    in_maps = make_in_maps(Xc, Xp, Xb, W_embed, b_embed, W_ih, W_hh,
                           b_ih, b_hh, W_bil)
    res = run_bass_kernel_spmd(_NC, in_maps, core_ids=list(range(N_CORES)))
    return gather(res.results)


# revision 13
# speedup vs baseline: 1.9072x; 1.0186x over previous
"""CPCNet forward on 8 Trainium2 NeuronCores (Bass/Tile).

Data-parallel over batch: each of the 8 cores processes 16 of the 128
batch elements end-to-end (embed GEMM -> GRU over 16 context windows ->
bilinear scoring), parameters replicated. No collectives needed.

The embed GEMM is memory-bound (~103 MB/core of f32 input). Host-side
transforms cut the HW time:
  1. X is pre-cast to bf16 on the host: halves HBM traffic (~51.6 MB/core,
     DMA roofline ~144 us at the 358 GB/s per-core HBM limit).
  2. X is pre-transposed on the host into k-major tiles [128 k-partitions,
     rows]: plain contiguous HWDGE DMAs feed the PE matmuls directly.
  3. W_embed chunks are zero-padded to M=128 so the PE's fast-weight-load
     path (NumWeights==128) pipelines LDWEIGHTS with the matmuls.

Single sync-ring DMA FIFO: W (2.2 MB), then Xcp (Xc+Xp, 4.3 MB) as quad
tiles, then Xb (43 MB) as 16 quad-chunk tiles of 2.5 MB + 1 pair tile.
A burst of warmup matmuls on W data runs before the stream so the PE's
HAM throttle promotes to full clock before real matmuls arrive. Per
k-chunk j (66), the PE accumulates W_j^T @ X_j^T into 6 persistent PSUM
banks (1 Ecp + 5 Eb). The GRU (fp32) runs one step per quad tile (two at
tile 8) so h is final a tile early; its elementwise chain runs on the
idle Vector engine reading gh straight from PSUM, with a single 3-block
sigmoid on ACT (the 1-z factor comes free as sigmoid(-z_preact) via a
4th negated gate block prepared on the host). The bilinear A-matrix and
positive products overlap the last tiles; the tail is 5 bank
evacuations, two batched negative-product multiplies, and a pipelined
ones-matmul reduction.

Row order inside E^T[100, 3072]: Xc 256 (s*16+b), Xp 256 (s*16+b),
Xb 2560 (nb*256 + s*16 + b). Output tmp/out layout is (s, p, b).
"""

import numpy as np

import concourse.bacc as bacc
import concourse.mybir as mybir
import concourse.tile as tile
from concourse.bass_utils import run_bass_kernel_spmd

N_CORES = 8
BC = 16          # batch per core
NE = 16          # context windows (gru seq len)
NB = 10          # negative samples
CT = 8400        # flattened window (21*400)
NK = 66          # k chunks of 128 (8400 zero-padded to 8448)
CTP = NK * 128
NQ = 16          # quad tiles (chunks 4t..4t+3); chunks 64,65 in a pair tile
E = 100          # embed dim == gru hidden
RCP = 2 * NE * BC          # 512 rows: Xc then Xp
RB = NB * NE * BC          # 2560 rows: nb*256 + s*16 + b
ROWS = RCP + RB            # 3072
NBB = RB // 512            # 5 Eb psum banks
TOT = NE * BC * (NB + 1)

F32 = mybir.dt.float32
BF16 = mybir.dt.bfloat16

# GRU steps run at half-tile boundaries of Xb quad tiles 1..8 (one step
# per ~2.2 us of PE work), so h is final by tile 8 and the last half of
# the stream is pure streaming with the PE catching up on any backlog.


def _emit(nc, tc, ctx):
    XcpT4 = nc.dram_tensor("XcpT4", [NQ, 128, 4 * RCP], BF16,
                           kind="ExternalInput").ap()
    XcpT2 = nc.dram_tensor("XcpT2", [128, 2 * RCP], BF16,
                           kind="ExternalInput").ap()
    XbT4 = nc.dram_tensor("XbT4", [NQ, 128, 4 * RB], BF16,
                          kind="ExternalInput").ap()
    XbT2 = nc.dram_tensor("XbT2", [128, 2 * RB], BF16,
                          kind="ExternalInput").ap()
    Wemb = nc.dram_tensor("Wemb", [128, NK * 128], BF16,
                          kind="ExternalInput").ap()
    bemb = nc.dram_tensor("bemb", [E, 1], F32, kind="ExternalInput").ap()
    Wih4T = nc.dram_tensor("Wih4T", [E, 400], F32, kind="ExternalInput").ap()
    WhhT = nc.dram_tensor("WhhT", [E, 300], F32, kind="ExternalInput").ap()
    bias5 = nc.dram_tensor("bias5", [E, 5], F32, kind="ExternalInput").ap()
    Wbil = nc.dram_tensor("Wbil", [E, NE * 128], BF16,
                          kind="ExternalInput").ap()
    ones = nc.dram_tensor("ones", [E, 1], mybir.dt.float32r,
                          kind="ExternalInput").ap()
    out_d = nc.dram_tensor("out", [1, TOT], F32, kind="ExternalOutput").ap()

    P = ctx.enter_context  # pools
    const = P(tc.tile_pool(name="const", bufs=1))
    xcp = P(tc.tile_pool(name="xcp", bufs=3))
    xbp = P(tc.tile_pool(name="xbp", bufs=4))
    psE = P(tc.tile_pool(name="psE", bufs=2, space="PSUM"))  # ecp/gp/Aall/rp
    psB = P(tc.tile_pool(name="psB", bufs=1, space="PSUM"))  # 5 Eb banks
    psS = P(tc.tile_pool(name="psS", bufs=1, space="PSUM"))  # gh
    small = P(tc.tile_pool(name="small", bufs=2))

    # W leads the sync FIFO (line rate, ~6us); other params ride gpsimd
    # SWDGE so the sync ring carries nothing but W + the X stream.
    W_sb = const.tile([128, NK * 128], BF16)
    nc.sync.dma_start(W_sb[:], Wemb[:])
    bemb_sb = const.tile([E, 1], F32)
    nc.gpsimd.dma_start(bemb_sb[:], bemb[:])
    Wih4T_sb = const.tile([E, 400], F32)
    nc.gpsimd.dma_start(Wih4T_sb[:], Wih4T[:])
    WhhT_sb = const.tile([E, 300], F32)
    nc.gpsimd.dma_start(WhhT_sb[:], WhhT[:])
    bias5_sb = const.tile([E, 5], F32)
    nc.gpsimd.dma_start(bias5_sb[:], bias5[:])
    Wbil_sb = const.tile([E, NE * 128], BF16)
    nc.gpsimd.dma_start(Wbil_sb[:], Wbil[:])
    ones_sb = const.tile([E, 1], mybir.dt.float32r)
    nc.gpsimd.dma_start(ones_sb[:], ones[:])

    ET = const.tile([E, ROWS], F32)                # all embeddings, transposed
    gi_sb = const.tile([E, NE * 4 * BC], F32)      # preacts, [s][r|z|n|mz]
    h = const.tile([E, BC], F32)                   # GRU hidden state (h^T)
    A_sb = const.tile([E, NE * BC], F32)           # bilinear A_s^T, s-major
    tmp_all = const.tile([E, TOT], mybir.dt.float32r)   # (s, p, b) layout
    out_sb = const.tile([1, TOT], F32)

    gi_v = gi_sb.rearrange("e (s g b) -> e s g b", s=NE, g=4)

    def gru_init():
        # gi preacts for all 16 steps in 4 gate matmuls (r, z, n, -z);
        # biases folded (r,z,mz get b_ih+b_hh; n gets b_ih only).
        nc.vector.memset(h[:], 0.0)
        for g in range(4):
            gp = psE.tile([E, NE * BC], F32, tag="pe", name="gp")
            nc.tensor.matmul(gp[:, :], Wih4T_sb[:, g * E:(g + 1) * E],
                             ET[:, 0:NE * BC], start=True, stop=True)
            nc.scalar.add(gi_v[:, :, g, :],
                          gp.rearrange("e (s b) -> e s b", s=NE),
                          bias5_sb[:, g:g + 1])

    def gru_step(s):
        # elementwise on the idle DVE reading gh straight from PSUM; one
        # 3-block sigmoid (r, z, 1-z) + one tanh on ACT.
        c0 = s * 4 * BC
        gh = psS.tile([E, 3 * BC], F32, tag="sp", name="gh")
        for g in range(3):
            nc.tensor.matmul(gh[:, g * BC:(g + 1) * BC],
                             WhhT_sb[:, g * E:(g + 1) * E], h[:],
                             start=True, stop=True)
        pre = small.tile([E, 4 * BC], F32, tag="pre", name="pre")
        nc.vector.tensor_add(pre[:, 0:2 * BC], gh[:, 0:2 * BC],
                             gi_sb[:, c0:c0 + 2 * BC])           # r,z preacts
        nc.vector.tensor_sub(pre[:, 2 * BC:3 * BC],
                             gi_sb[:, c0 + 3 * BC:c0 + 4 * BC],
                             gh[:, BC:2 * BC])                   # -(z preact)
        nc.vector.tensor_scalar_add(pre[:, 3 * BC:4 * BC],
                                    gh[:, 2 * BC:3 * BC],
                                    bias5_sb[:, 4:5])            # gh_n + b_hn
        sg = small.tile([E, 3 * BC], F32, tag="sg", name="sg")
        nc.scalar.activation(sg[:], pre[:, 0:3 * BC],
                             mybir.ActivationFunctionType.Sigmoid)
        t1 = small.tile([E, BC], F32, tag="t1", name="t1")
        nc.vector.tensor_mul(t1[:], sg[:, 0:BC], pre[:, 3 * BC:4 * BC])
        t2 = small.tile([E, BC], F32, tag="t2", name="t2")
        nc.vector.tensor_add(t2[:], t1[:],
                             gi_sb[:, c0 + 2 * BC:c0 + 3 * BC])
        zh = small.tile([E, BC], F32, tag="zh", name="zh")
        nc.vector.tensor_mul(zh[:], sg[:, BC:2 * BC], h[:])      # z*h_old
        n = small.tile([E, BC], F32, tag="n", name="n")
        nc.scalar.activation(n[:], t2[:], mybir.ActivationFunctionType.Tanh)
        on = small.tile([E, BC], F32, tag="on", name="on")
        nc.vector.tensor_mul(on[:], sg[:, 2 * BC:3 * BC], n[:])  # (1-z)*n
        nc.vector.tensor_add(h[:], on[:], zh[:])                 # new h

    # ---- PE warmup: ~4us of matmuls on W data so the HAM throttle
    # promotes to full clock before the real stream arrives ----
    eb = [psB.tile([128, 512], F32, name=f"eb{i}") for i in range(NBB)]
    for w in range(16):
        nc.tensor.matmul(eb[0][:, :], W_sb[:, (w % 4) * 128:(w % 4 + 1) * 128],
                         W_sb[:, 0:512], start=True, stop=True)

    # ---- Xcp stream (quad tiles, head of the FIFO after W) ----
    ecp = psE.tile([128, RCP], F32, tag="pe", name="ecp")

    def ecp_mms(xt, j, un):
        for u in range(un):
            nc.tensor.matmul(ecp[:, :], W_sb[:, (j + u) * 128:(j + u + 1) * 128],
                             xt[:, u * RCP:(u + 1) * RCP],
                             start=(j + u == 0), stop=(j + u == NK - 1),
                             skip_group_check=True)

    for t in range(NQ):
        xt = xcp.tile([128, 4 * RCP], BF16, tag="xcp", name="xcp_t")
        nc.sync.dma_start(xt[:], XcpT4[t])
        ecp_mms(xt, 4 * t, 4)
    xtc2 = xcp.tile([128, 2 * RCP], BF16, tag="xcp2", name="xcp2_t")
    nc.sync.dma_start(xtc2[:], XcpT2[:])
    ecp_mms(xtc2, 64, 2)

    # ---- Xb stream (same FIFO): 16 quad tiles + 1 pair tile; GRU steps
    # at half-tile boundaries of tiles 1..8 ----
    def emb_mms(xt, j, un, cbase=0):
        for u in range(un):
            c = cbase + u
            for blk in range(NBB):
                nc.tensor.matmul(eb[blk][:, :],
                                 W_sb[:, (j + u) * 128:(j + u + 1) * 128],
                                 xt[:, c * RB + blk * 512:
                                        c * RB + (blk + 1) * 512],
                                 start=(j + u == 0), stop=(j + u == NK - 1),
                                 skip_group_check=True)

    h_bf = const.tile([E, BC], BF16)
    tmp_v = tmp_all.rearrange("e (s p b) -> e s p b", s=NE, p=NB + 1)
    A_v = A_sb.rearrange("e (s b) -> e s b", s=NE)

    for t in range(NQ):
        xt = xbp.tile([128, 4 * RB], BF16, tag="xb", name="xb_t")
        nc.sync.dma_start(xt[:], XbT4[t])
        emb_mms(xt, 4 * t, 2)
        if 1 <= t <= 8:
            gru_step(2 * (t - 1))
        emb_mms(xt, 4 * t + 2, 2, cbase=2)
        if t == 0:
            nc.scalar.add(ET[:, 0:RCP], ecp[0:E, :], bemb_sb[:, 0:1])
            gru_init()
        elif t <= 8:
            gru_step(2 * (t - 1) + 1)
        elif t == 9:
            # h final: bilinear A via bf16 M=128-padded Wbil (FWL keeps
            # LDWEIGHTS pipelined), then positive products -- all while
            # tiles 10+ stream.
            nc.vector.tensor_copy(h_bf[:], h[:])
            A_ps = psE.tile([128, NE * BC], F32, tag="pe", name="A_ps")
            for s in range(NE):
                nc.tensor.matmul(A_ps[:, s * BC:(s + 1) * BC],
                                 Wbil_sb[:, s * 128:(s + 1) * 128], h_bf[:],
                                 start=True, stop=True)
            nc.scalar.copy(A_sb[:], A_ps[0:E, :])
            nc.vector.tensor_mul(tmp_v[:, :, 0, :],
                                 ET[:, NE * BC:2 * NE * BC]
                                 .rearrange("e (s b) -> e s b", s=NE),
                                 A_v)

    xt2 = xbp.tile([128, 2 * RB], BF16, tag="xb2", name="xb2_t")
    nc.sync.dma_start(xt2[:], XbT2[:])
    emb_mms(xt2, 64, 2)

    # ---- evacuate Eb banks (ACT for 0-2, DVE for 3-4), then the
    # negative products in two batched multiplies (nb 0-3, nb 4-9) ----
    for blk in range(3):
        nc.scalar.add(ET[:, RCP + blk * 512:RCP + (blk + 1) * 512],
                      eb[blk][0:E, :], bemb_sb[:, 0:1])
    for blk in range(3, NBB):
        nc.vector.tensor_scalar_add(
            ET[:, RCP + blk * 512:RCP + (blk + 1) * 512],
            eb[blk][0:E, :], bemb_sb[:, 0:1])
    Eb_v = ET[:, RCP:ROWS].rearrange("e (nb s b) -> e s nb b", nb=NB, s=NE)
    nc.vector.tensor_mul(tmp_v[:, :, 1:5, :], Eb_v[:, :, 0:4, :],
                         A_v.unsqueeze(2).broadcast_to([E, NE, 4, BC]))
    nc.vector.tensor_mul(tmp_v[:, :, 5:NB + 1, :], Eb_v[:, :, 4:NB, :],
                         A_v.unsqueeze(2).broadcast_to([E, NE, 6, BC]))

    # ---- ones-matmul reduction over E, pipelined via psE rotation;
    # copies alternate ACT/DVE so consecutive chunks overlap ----
    for i, c0 in enumerate(range(0, TOT, 512)):
        w = min(512, TOT - c0)
        rp = psE.tile([1, 512], F32, tag="pe", name="rp")
        nc.tensor.matmul(rp[0:1, 0:w], ones_sb[:, 0:1], tmp_all[:, c0:c0 + w],
                         start=True, stop=True)
        if i % 2 == 0:
            nc.scalar.copy(out_sb[:, c0:c0 + w], rp[0:1, 0:w])
        else:
            nc.vector.tensor_copy(out_sb[:, c0:c0 + w], rp[0:1, 0:w])
    nc.sync.dma_start(out_d[:], out_sb[:])


def build():
    import contextlib
    nc = bacc.Bacc("TRN2", target_bir_lowering=False, debug=False,
                   enable_asserts=False, num_devices=N_CORES)
    with tile.TileContext(nc) as tc:
        with contextlib.ExitStack() as ctx:
            _emit(nc, tc, ctx)
    nc.compile()
    return nc


_NC = None
_PREP = None


def _get_prep():
    """jax-cpu jitted reorder + bf16 cast + k-major transpose (XLA's blocked
    multithreaded transpose; a naive numpy transpose of 826 MB is too slow)."""
    global _PREP
    if _PREP is None:
        import jax
        import jax.numpy as jnp
        cpu = jax.devices("cpu")[0]

        def fmt(x, R):
            x = jnp.pad(x, ((0, 0), (0, CTP - CT))).astype(jnp.bfloat16)
            x4 = x[:, :NQ * 4 * 128].reshape(R, NQ, 4, 128)
            xT4 = jnp.transpose(x4, (1, 3, 2, 0)).reshape(NQ, 128, 4 * R)
            x2 = x[:, NQ * 4 * 128:].reshape(R, 2, 128)
            xT2 = jnp.transpose(x2, (2, 1, 0)).reshape(128, 2 * R)
            return xT4, xT2

        def f(xc, xp, xb):
            # xc, xp: [BC, NE, CT]; xb: [BC, NE, NB, CT] (f32)
            xcp = jnp.concatenate([
                jnp.transpose(xc, (1, 0, 2)).reshape(NE * BC, CT),
                jnp.transpose(xp, (1, 0, 2)).reshape(NE * BC, CT)], axis=0)
            xbr = jnp.transpose(xb, (2, 1, 0, 3)).reshape(RB, CT)
            return fmt(xcp, RCP) + fmt(xbr, RB)

        with jax.default_device(cpu):
            jf = jax.jit(f)

        def prep(xc, xp, xb):
            with jax.default_device(cpu):
                return tuple(np.asarray(v) for v in jf(xc, xp, xb))
        _PREP = prep
    return _PREP


def make_in_maps(Xc, Xp, Xb, W_embed, b_embed, W_ih, W_hh, b_ih, b_hh, W_bil):
    B = Xc.shape[0]
    Xc_r = np.ascontiguousarray(Xc, np.float32).reshape(B, NE, CT)
    Xp_r = np.ascontiguousarray(Xp, np.float32).reshape(B, NE, CT)
    Xb_r = np.ascontiguousarray(Xb, np.float32).reshape(B, NE, NB, CT)

    import ml_dtypes
    W_embed = np.ascontiguousarray(W_embed, np.float32)
    # chunk j at cols [j*128, j*128+E); M zero-padded to 128 so the PE's
    # fast-weight-load path (NumWeights==128) is enabled.
    W_ch = np.zeros((128, NK * 128), np.float32)
    for j in range(NK):
        kj = min(128, CT - j * 128)
        W_ch[:kj, j * 128:j * 128 + E] = W_embed[j * 128:j * 128 + kj]
    W_ch = W_ch.astype(ml_dtypes.bfloat16)
    bemb = np.ascontiguousarray(b_embed, np.float32).reshape(E, 1)
    WihT = np.ascontiguousarray(W_ih.T, np.float32)          # [100, 300]
    Wih4T = np.concatenate([WihT, -WihT[:, E:2 * E]], axis=1)  # [100, 400]
    WhhT = np.ascontiguousarray(W_hh.T, np.float32)
    bz = b_ih[E:2 * E] + b_hh[E:2 * E]
    bias5 = np.stack([b_ih[0:E] + b_hh[0:E], bz, b_ih[2 * E:3 * E],
                      -bz, b_hh[2 * E:3 * E]], axis=1).astype(np.float32)
    # W_bil blocks M-padded to 128 + bf16 so the A matmuls get FWL.
    Wbil_t = np.transpose(W_bil, (1, 0, 2))                  # [H, NE, E]
    Wbil_r = np.zeros((E, NE * 128), np.float32)
    for s in range(NE):
        Wbil_r[:, s * 128:s * 128 + E] = Wbil_t[:, s, :]
    Wbil_r = Wbil_r.astype(ml_dtypes.bfloat16)
    ones = np.ones((E, 1), np.float32)

    shared = dict(Wemb=W_ch, bemb=bemb, Wih4T=Wih4T, WhhT=WhhT,
                  bias5=bias5, Wbil=Wbil_r, ones=ones)
    prep = _get_prep()
    in_maps = []
    for c in range(N_CORES):
        sl = slice(c * BC, (c + 1) * BC)
        xcpT4, xcpT2, xbT4, xbT2 = prep(Xc_r[sl], Xp_r[sl], Xb_r[sl])
        in_maps.append(dict(XcpT4=xcpT4, XcpT2=xcpT2, XbT4=xbT4, XbT2=xbT2,
                            **shared))
    return in_maps


def gather(results):
    outs = []
    for c in range(N_CORES):
        o = results[c]["out"].reshape(NE, NB + 1, BC)       # [s, p, b]
        outs.append(np.transpose(o, (2, 0, 1)))             # [b, s, p]
    return np.concatenate(outs, axis=0).astype(np.float32)  # [128, 16, 11]


def kernel(Xc, Xp, Xb, W_embed, b_embed, W_ih, W_hh, b_ih, b_hh, W_bil):
    global _NC
    if _NC is None:
        _NC = build()
    in_maps = make_in_maps(Xc, Xp, Xb, W_embed, b_embed, W_ih, W_hh,
                           b_ih, b_hh, W_bil)
    res = run_bass_kernel_spmd(_NC, in_maps, core_ids=list(range(N_CORES)))
    return gather(res.results)
